# revision 1
# baseline (speedup 1.0000x reference)
"""DeepSeekV2-style MLA attention forward on 8 Trainium2 NeuronCores.

Sharding: 2-way data-parallel over batch x 4-way tensor-parallel over heads
(4 heads per core). The shared low-rank q_a/kv_a projections are sharded
over TOKENS within each batch's TP group: each core projects+rmsnorms its
own quarter of the sequence fully locally, then one AllGather per tensor
(normalized q_a, and c_kv_norm|k_rot combined) replicates them. o_proj
partial outputs are summed on the host (TP unshard).

Layout convention on device: activations live transposed as [feature, token]
so that every matmul is out^T[f_out, t] = lhsT(W^T tile).T @ rhs(x^T tile),
with weights pre-transposed on the host. All matmuls use the f32r
(reduced-precision fp32) PE path: 4x faster than fp32, ~1.4e-4 rel err.

Attention: scores are computed transposed s^T[k, q] (k on partitions), exp'd
on ScalarE without max-subtraction (scores are provably small here), masked
on the causal diagonal blocks, then AV uses p^T as the 512-wide moving
operand (out^T[dv, q]) with the softmax denominator from a ones-column
matmul; o_proj is fused into the same q-tile loop.
"""
import math
import sys

import numpy as np

try:
    import concourse.bass as bass  # noqa: F401
except ImportError:  # pragma: no cover
    sys.path.insert(0, "/opt/trn_rl_repo")

import concourse.bass as bass
import concourse.tile as tile
from concourse import bacc, mybir
from concourse.bass_utils import run_bass_kernel_spmd

# ---- problem dims (hardcoded per contest contract) ----
B, S, HID = 2, 2048, 2048
NH = 16
DN, DR, DV = 128, 64, 128
QD = DN + DR                       # 192
QLR, KVLR = 1536, 512
EPS = 1e-6
ROPE_BASE = 10000.0
SCALE = 1.0 / math.sqrt(QD)

N_CORES = 8
TPG = 4                            # TP group size (cores per batch)
HPC = NH // TPG                    # heads per core = 4

F32 = mybir.dt.float32
F32R = mybir.dt.float32r
I32 = mybir.dt.int32

NKV = KVLR + DR                    # 576 kv_a rows
T_TILE = 512                       # token tile (free dim)
NT = S // T_TILE                   # 4 token tiles
KB = S // 128                      # 16 key tiles of 128

NFO_KV = KVLR // 128               # 4
NFO_QA = QLR // 128                # 12
NHI = HID // 128                   # 16

TWO_PI = 2.0 * math.pi
MAGIC = np.float32(1.5 * 2**23)    # round-to-nearest-int magic constant

REPLICA_GROUPS = [[0, 1, 2, 3], [4, 5, 6, 7]]


def _cody_waite_consts():
    def trunc12(x):
        return np.frombuffer(
            (np.frombuffer(np.float32(x).tobytes(), np.uint32)
             & np.uint32(0xFFFFF000)).tobytes(), np.float32)[0]
    c1 = trunc12(np.float64(TWO_PI))
    c2 = trunc12(np.float64(TWO_PI) - np.float64(c1))
    c3 = np.float32(np.float64(TWO_PI) - np.float64(c1) - np.float64(c2))
    return float(c1), float(c2), float(c3)


CW1, CW2, CW3 = _cody_waite_consts()

_BUILD_CACHE = {}


def build_kernel(debug=False):
    key = bool(debug)
    if key in _BUILD_CACHE:
        return _BUILD_CACHE[key]

    nc = bacc.Bacc("TRN2", target_bir_lowering=False, debug=False,
                   num_devices=N_CORES)

    def din(name, shape, dt=F32R):
        return nc.dram_tensor(name, list(shape), dt, kind="ExternalInput").ap()

    # ---- per-core external inputs ----
    xTl = din("xTl", [HID, T_TILE])                # hidden^T, LOCAL tokens
    w_qaT = din("w_qaT", [HID, QLR])
    w_kvaT = din("w_kvaT", [HID, NKV])
    w_qbT = din("w_qbT", [QLR, HPC * QD])          # cols: nope h0..h3 | rope h0..h3
    w_kvb_nT = din("w_kvb_nT", [KVLR, HPC * DN])   # k_nope cols by head
    w_kvb_vT = din("w_kvb_vT", [KVLR, HPC * DV])   # v cols by head
    w_oT = din("w_oT", [HPC * DV, HID])
    b_qa = din("b_qa", [128, NFO_QA], F32)
    b_kva = din("b_kva", [128, 5], F32)            # 576 padded to 640
    ln_qa = din("ln_qa", [128, NFO_QA], F32)
    ln_kva = din("ln_kva", [128, NFO_KV], F32)
    pos = din("pos", [1, S], I32)                  # full positions (for q rope)
    pos_l = din("pos_l", [1, T_TILE], I32)         # local positions (k_pe rope)
    inv_freq = din("inv_freq", [128, 1], F32)      # rope inv freqs, 4x repeated
    p128 = din("p128", [128, 128])                 # blockdiag(rotT, rotT) f32r
    ones_col = din("ones_col", [128, 1])           # f32r ones (sum matmuls)
    masks = din("masks", [4, 128, T_TILE])         # f32r causal diag masks

    out = nc.dram_tensor("out", [S, HID], F32, kind="ExternalOutput").ap()

    # ---- DRAM intermediates ----
    ikind = "ExternalOutput" if debug else "Internal"

    def dmid(name, shape, dt):
        return nc.dram_tensor(name, list(shape), dt, kind=ikind).ap()

    # collective in/out tensors must stay Internal (cannot be IO)
    qa_sh = nc.dram_tensor("qa_sh", [QLR, T_TILE], F32R).ap()
    qa_all = nc.dram_tensor("qa_all", [NT, QLR, T_TILE], F32R).ap()
    ckv_sh = nc.dram_tensor("ckv_sh", [NKV, T_TILE], F32R).ap()  # norm| k_rot
    ckv_all = nc.dram_tensor("ckv_all", [NT, NKV, T_TILE], F32R).ap()

    qT_nope = dmid("qT_nope", [HPC * DN, S], F32R)
    qT_rope = dmid("qT_rope", [HPC * DR, S], F32R)  # rope'd, by head
    if debug:
        dbg_kn = dmid("dbg_kn", [HPC * DN, S], F32R)
        dbg_v = dmid("dbg_v", [KB, 128, HPC, DV], F32R)
        dbg_krot = dmid("dbg_krot", [DR, S], F32R)
        attn_T = dmid("attn_T", [HPC * DV, S], F32R)
        dbg_sin = dmid("dbg_sin", [128, S], F32)
        dbg_cos = dmid("dbg_cos", [128, S], F32)

    with tile.TileContext(nc) as tc:
        with tc.tile_pool(name="const", bufs=1) as constp:
            bqa_t = constp.tile([128, NFO_QA], F32)
            nc.sync.dma_start(bqa_t[:], b_qa[:])
            bkva_t = constp.tile([128, 5], F32)
            nc.sync.dma_start(bkva_t[:], b_kva[:])
            lnqa_t = constp.tile([128, NFO_QA], F32)
            nc.sync.dma_start(lnqa_t[:], ln_qa[:])
            lnkva_t = constp.tile([128, NFO_KV], F32)
            nc.sync.dma_start(lnkva_t[:], ln_kva[:])
            ones_t = constp.tile([128, 1], F32R)
            nc.sync.dma_start(ones_t[:], ones_col[:])
            p128_t = constp.tile([128, 128], F32R)
            nc.sync.dma_start(p128_t[:], p128[:])
            ivf_t = constp.tile([128, 1], F32)
            nc.sync.dma_start(ivf_t[:], inv_freq[:])

            def rope_tables(pos_ap, n, cos_dst, sin_dst, rp, tag):
                """Build cos/sin [128, n] tables from int32 positions [1, n]."""
                pos_i = rp.tile([1, n], I32, name=f"pos_i_{tag}")
                nc.sync.dma_start(pos_i[:], pos_ap[:])
                pos_f = rp.tile([1, n], F32, name=f"pos_f_{tag}")
                nc.vector.tensor_copy(pos_f[:], pos_i[:])
                pos_b = rp.tile([128, n], F32, name=f"pos_b_{tag}")
                nc.gpsimd.partition_broadcast(pos_b[:], pos_f[:])
                freqs = rp.tile([128, n], F32, name=f"freqs_{tag}")
                nc.vector.tensor_scalar_mul(freqs[:], pos_b[:], ivf_t[:])
                kr = rp.tile([128, n], F32, name=f"kr_{tag}")
                nc.vector.tensor_scalar(kr[:], freqs[:], 1.0 / TWO_PI,
                                        float(MAGIC), mybir.AluOpType.mult,
                                        mybir.AluOpType.add)
                nc.vector.tensor_scalar_sub(kr[:], kr[:], float(MAGIC))
                red = rp.tile([128, n], F32, name=f"red_{tag}")
                nc.vector.cody_waite_cascade(red[:], freqs[:], kr[:],
                                             CW1, CW2, CW3)
                nc.scalar.activation(sin_dst, red[:],
                                     mybir.ActivationFunctionType.Sin)
                redc = rp.tile([128, n], F32, name=f"redc_{tag}")
                nc.vector.add_range_wrap(redc[:], red[:], math.pi / 2.0,
                                         math.pi, TWO_PI)
                nc.scalar.activation(cos_dst, redc[:],
                                     mybir.ActivationFunctionType.Sin)

            # ---------- phase A: local-token q_a / kv_a + rmsnorm + k rope --
            xTl_r = xTl.rearrange("(hi p) s -> p hi s", p=128)
            w_qaT_r = w_qaT.rearrange("(hi p) f -> p hi f", p=128)
            w_kvaT_r = w_kvaT.rearrange("(hi p) f -> p hi f", p=128)
            qa_sh_r = qa_sh.rearrange("(f p) s -> p f s", p=128)

            with nc.named_scope("proj_a"), \
                 tc.tile_pool(name="ap_", bufs=1) as ap_, \
                 tc.tile_pool(name="wa", bufs=2) as wap, \
                 tc.tile_pool(name="va", bufs=1) as vap, \
                 tc.tile_pool(name="pa", bufs=3, space="PSUM") as pap, \
                 tc.tile_pool(name="ssp", bufs=1, space="PSUM") as ssp:
                # local rope tables for k_pe
                cos_l = ap_.tile([128, T_TILE], F32)
                sin_l = ap_.tile([128, T_TILE], F32)
                rope_tables(pos_l, T_TILE, cos_l[:], sin_l[:], ap_, "loc")

                xa = ap_.tile([128, NHI, T_TILE], F32R)
                nc.sync.dma_start(xa[:], xTl_r[:])
                val_qa = ap_.tile([128, NFO_QA, T_TILE], F32)
                val_kv = ap_.tile([128, 5, T_TILE], F32)
                ss_qa = ssp.tile([1, T_TILE], F32, name="ss_qa")
                ss_kv = ssp.tile([1, T_TILE], F32, name="ss_kv")
                for proj in range(2):
                    nfo = NFO_QA if proj == 0 else 5
                    wsrc = w_qaT_r if proj == 0 else w_kvaT_r
                    bias_t = bqa_t if proj == 0 else bkva_t
                    vdst = val_qa if proj == 0 else val_kv
                    for fo in range(nfo):
                        m = 128 if not (proj == 1 and fo == 4) else 64
                        wt = wap.tile([128, NHI, 128], F32R, tag="wt")
                        nc.sync.dma_start(
                            wt[:, :, :m], wsrc[:, :, fo * 128:fo * 128 + m])
                        ps = pap.tile([m, T_TILE], F32, tag="acc")
                        for hi in range(NHI):
                            nc.tensor.matmul(
                                ps[:], wt[:, hi, :m], xa[:, hi, :],
                                start=(hi == 0), stop=(hi == NHI - 1))
                        nc.vector.tensor_scalar_add(
                            vdst[:m, fo, :], ps[:], bias_t[:m, fo:fo + 1])
                        if not (proj == 1 and fo == 4):
                            sq = vap.tile([128, T_TILE], F32R, tag="sq")
                            nc.vector.tensor_tensor(
                                sq[:], vdst[:, fo, :], vdst[:, fo, :],
                                mybir.AluOpType.mult)
                            sst = ss_qa if proj == 0 else ss_kv
                            nc.tensor.matmul(
                                sst[:], ones_t[:], sq[:],
                                start=(fo == 0),
                                stop=(fo == nfo - 1 - (proj == 1)))
                # rstd (fully local), normalize, then AllGather
                for proj in range(2):
                    d = QLR if proj == 0 else KVLR
                    nfo = NFO_QA if proj == 0 else NFO_KV
                    sst = ss_qa if proj == 0 else ss_kv
                    ln_t = lnqa_t if proj == 0 else lnkva_t
                    vsrc = val_qa if proj == 0 else val_kv
                    ms = vap.tile([1, T_TILE], F32, tag="ms")
                    nc.vector.tensor_scalar(
                        ms[:], sst[:], 1.0 / d, EPS,
                        mybir.AluOpType.mult, mybir.AluOpType.add)
                    std = vap.tile([1, T_TILE], F32, tag="std")
                    nc.scalar.activation(std[:], ms[:],
                                         mybir.ActivationFunctionType.Sqrt)
                    rstd = vap.tile([1, T_TILE], F32, tag="rstd")
                    nc.vector.reciprocal(rstd[:], std[:])
                    rstd_b = vap.tile([128, T_TILE], F32, tag="rstdb")
                    nc.gpsimd.partition_broadcast(rstd_b[:], rstd[:])
                    for f in range(nfo):
                        nrm = vap.tile([128, T_TILE], F32R, tag="nrm")
                        nc.vector.scalar_tensor_tensor(
                            nrm[:], vsrc[:, f, :], ln_t[:, f:f + 1],
                            rstd_b[:],
                            mybir.AluOpType.mult, mybir.AluOpType.mult)
                        if proj == 0:
                            nc.sync.dma_start(qa_sh_r[:, f, :], nrm[:])
                        else:
                            nc.sync.dma_start(
                                ckv_sh[f * 128:(f + 1) * 128, :], nrm[:])
                # k_pe rope (local tokens) -> ckv_sh rows 512..576
                kpe = vap.tile([64, T_TILE], F32R, tag="kpe")
                nc.vector.tensor_copy(kpe[:], val_kv[0:64, 4, :])
                rps = pap.tile([64, T_TILE], F32, tag="rotk")
                nc.tensor.matmul(rps[:], p128_t[0:64, 0:64], kpe[:],
                                 start=True, stop=True)
                tmp = vap.tile([64, T_TILE], F32, tag="tmpk")
                nc.vector.tensor_tensor(tmp[:], cos_l[0:64, :], kpe[:],
                                        mybir.AluOpType.mult)
                rot = vap.tile([64, T_TILE], F32, tag="rotk2")
                nc.vector.tensor_tensor(rot[:], sin_l[0:64, :], rps[:],
                                        mybir.AluOpType.mult)
                kro = vap.tile([64, T_TILE], F32R, tag="kro")
                nc.vector.tensor_tensor(kro[:], tmp[:], rot[:],
                                        mybir.AluOpType.add)
                nc.sync.dma_start(ckv_sh[KVLR:KVLR + DR, :], kro[:])

                # gather: kv first (small, unblocks kv_b), then q_a
                nc.gpsimd.collective_compute(
                    "AllGather", mybir.AluOpType.bypass,
                    replica_groups=REPLICA_GROUPS,
                    ins=[ckv_sh[:]], outs=[ckv_all[:]])
                nc.gpsimd.collective_compute(
                    "AllGather", mybir.AluOpType.bypass,
                    replica_groups=REPLICA_GROUPS,
                    ins=[qa_sh[:]], outs=[qa_all[:]])

            # ---------- phase R: full rope cos/sin tables (for q) ----------
            cos_t = constp.tile([128, NT, T_TILE], F32)
            sin_t = constp.tile([128, NT, T_TILE], F32)
            with nc.named_scope("rope_tables"), \
                 tc.tile_pool(name="ropep", bufs=1) as rp:
                rope_tables(pos, S, cos_t.rearrange("p n t -> p (n t)"),
                            sin_t.rearrange("p n t -> p (n t)"), rp, "full")
                if debug:
                    nc.sync.dma_start(dbg_sin[:],
                                      sin_t.rearrange("p n t -> p (n t)"))
                    nc.sync.dma_start(dbg_cos[:],
                                      cos_t.rearrange("p n t -> p (n t)"))

            # C-phase resident tiles, direct-written by phase B
            kv_res = tc.alloc_tile_pool(name="kv_res", bufs=1)
            kn_sb = kv_res.tile([128, HPC, S], F32R, name="kn_sb")
            vh_sb = kv_res.tile([128, HPC, KB, DV], F32R, name="vh_sb")
            krot_sb = kv_res.tile([64, S], F32R, name="krot_sb")

            # ---------- phase B: q_b / kv_b / q rope ----------
            qT_nope_r = qT_nope.rearrange("(f p) s -> p f s", p=128)
            qT_rope_r2 = qT_rope.rearrange("(f p) s -> p f s", p=128)
            w_qbT_r = w_qbT.rearrange("(fi p) f -> p fi f", p=128)
            w_kvb_nT_r = w_kvb_nT.rearrange("(fi p) f -> p fi f", p=128)
            w_kvb_vT_r = w_kvb_vT.rearrange("(fi p) f -> p fi f", p=128)

            NQB = HPC * QD // 128  # 6 output tiles (4 nope + 2 rope-pair)

            with nc.named_scope("proj_b"), \
                 tc.tile_pool(name="wb", bufs=2) as wbp, \
                 tc.tile_pool(name="rhb", bufs=2) as rhbp, \
                 tc.tile_pool(name="evb", bufs=2) as evbp, \
                 tc.tile_pool(name="pb", bufs=2, space="PSUM") as pbp:
                wv_t = wbp.tile([128, NFO_KV, HPC * DV], F32R, name="wv_t",
                                bufs=1)
                nc.gpsimd.dma_start(wv_t[:], w_kvb_vT_r[:])
                for t in range(NT):
                    tsl = slice(t * T_TILE, (t + 1) * T_TILE)
                    qa_rhs = rhbp.tile([128, NFO_QA, T_TILE], F32R,
                                       tag="qarhs")
                    ckv_rhs = rhbp.tile([128, NFO_KV, T_TILE], F32R,
                                        tag="ckvrhs", bufs=1)
                    for f in range(NFO_QA):
                        nc.gpsimd.dma_start(
                            qa_rhs[:, f, :],
                            qa_all[t, f * 128:(f + 1) * 128, :])
                    for f in range(NFO_KV):
                        nc.gpsimd.dma_start(
                            ckv_rhs[:, f, :],
                            ckv_all[t, f * 128:(f + 1) * 128, :])
                    # k_rot arrives via the ckv AllGather
                    nc.gpsimd.dma_start(
                        krot_sb[:, tsl], ckv_all[t, KVLR:KVLR + DR, :])

                    # q_b: 4 nope tiles then 2 rope-pair tiles
                    for fo in range(NQB):
                        wt = wbp.tile([128, NFO_QA, 128], F32R, tag="wqb")
                        nc.gpsimd.dma_start(
                            wt[:], w_qbT_r[:, :, fo * 128:(fo + 1) * 128])
                        ps = pbp.tile([128, T_TILE], F32, tag="qb")
                        for fi in range(NFO_QA):
                            nc.tensor.matmul(ps[:], wt[:, fi, :],
                                             qa_rhs[:, fi, :],
                                             start=(fi == 0),
                                             stop=(fi == NFO_QA - 1))
                        if fo < HPC:  # nope
                            ev = evbp.tile([128, T_TILE], F32R, tag="evr")
                            nc.scalar.activation(
                                ev[:], ps[:],
                                mybir.ActivationFunctionType.Copy)
                            nc.sync.dma_start(qT_nope_r[:, fo, tsl], ev[:])
                        else:  # rope pair: rows = heads (2j, 2j+1)
                            qpe = evbp.tile([128, T_TILE], F32R, tag="evr")
                            nc.scalar.activation(
                                qpe[:], ps[:],
                                mybir.ActivationFunctionType.Copy)
                            rps = pbp.tile([128, T_TILE], F32, tag="rot",
                                           bufs=1)
                            nc.tensor.matmul(rps[:], p128_t[:], qpe[:],
                                             start=True, stop=True)
                            tmp = evbp.tile([128, T_TILE], F32, tag="tmp")
                            nc.vector.tensor_tensor(
                                tmp[:], cos_t[:, t, :], qpe[:],
                                mybir.AluOpType.mult)
                            rot = evbp.tile([128, T_TILE], F32, tag="rot2")
                            nc.vector.tensor_tensor(
                                rot[:], sin_t[:, t, :], rps[:],
                                mybir.AluOpType.mult)
                            qro = evbp.tile([128, T_TILE], F32R, tag="evr2")
                            nc.vector.tensor_tensor(
                                qro[:], tmp[:], rot[:], mybir.AluOpType.add)
                            j = fo - HPC
                            nc.sync.dma_start(qT_rope_r2[:, j, tsl], qro[:])

                    # kv_b nope -> straight into C-resident kn_sb
                    for fo in range(HPC):
                        wt = wbp.tile([128, NFO_KV, 128], F32R, tag="wkn")
                        nc.gpsimd.dma_start(
                            wt[:], w_kvb_nT_r[:, :, fo * 128:(fo + 1) * 128])
                        ps = pbp.tile([128, T_TILE], F32, tag="qb")
                        for fi in range(NFO_KV):
                            nc.tensor.matmul(ps[:], wt[:, fi, :],
                                             ckv_rhs[:, fi, :],
                                             start=(fi == 0),
                                             stop=(fi == NFO_KV - 1))
                        nc.scalar.activation(
                            kn_sb[:, fo, tsl], ps[:],
                            mybir.ActivationFunctionType.Copy)

                    # v (un-transposed) -> straight into C-resident vh_sb
                    for ts in range(T_TILE // 128):
                        kb = t * 4 + ts
                        ps = pbp.tile([128, HPC * DV], F32, tag="vps",
                                      bufs=2)
                        for fi in range(NFO_KV):
                            nc.tensor.matmul(
                                ps[:],
                                ckv_rhs[:, fi, ts * 128:(ts + 1) * 128],
                                wv_t[:, fi, :],
                                start=(fi == 0), stop=(fi == NFO_KV - 1))
                        nc.scalar.activation(
                            vh_sb[:, :, kb, :],
                            ps[:].rearrange("p (h d) -> p h d", h=HPC),
                            mybir.ActivationFunctionType.Copy)

            # ---------- phase C: attention + fused o_proj ----------
            w_oT_r = w_oT.rearrange("(fs p) hid -> p fs hid", p=128)
            qT_rope_r = qT_rope.rearrange("(f p) s -> p f s", p=64)
            with nc.named_scope("attn"), \
                 tc.tile_pool(name="cw", bufs=1) as cwp, \
                 tc.tile_pool(name="qrh", bufs=3) as qrhp, \
                 tc.tile_pool(name="pt", bufs=4) as ptp, \
                 tc.tile_pool(name="ao", bufs=2) as aop, \
                 tc.tile_pool(name="oe", bufs=3) as oep, \
                 tc.tile_pool(name="sps", bufs=2, space="PSUM") as spsp, \
                 tc.tile_pool(name="avs", bufs=2, space="PSUM") as avsp, \
                 tc.tile_pool(name="lps", bufs=2, space="PSUM") as lpsp, \
                 tc.tile_pool(name="pos_", bufs=2, space="PSUM") as posp:
                masks_t = cwp.tile([128, 4, T_TILE], F32R, name="masks_t")
                nc.sync.dma_start(masks_t[:], masks.rearrange("j p t -> p j t"))
                wo_sb = cwp.tile([128, HPC, HID], F32R, name="wo_sb")
                nc.sync.dma_start(wo_sb[:], w_oT_r[:])
                if debug:
                    nc.sync.dma_start(
                        dbg_kn.rearrange("(f p) s -> p f s", p=128)[:],
                        kn_sb[:])
                    nc.sync.dma_start(
                        dbg_v.rearrange("kb p h d -> p h kb d"), vh_sb[:])
                    nc.sync.dma_start(dbg_krot[:], krot_sb[:])
                for qt in range(NT):
                    qsl = slice(qt * T_TILE, (qt + 1) * T_TILE)
                    at_full = aop.tile([128, HPC, T_TILE], F32R, tag="atf")
                    nkb = 4 * qt + 4
                    for h in range(HPC):
                        qn_rhs = qrhp.tile([128, T_TILE], F32R, tag="qn")
                        nc.sync.dma_start(qn_rhs[:], qT_nope_r[:, h, qsl])
                        qr_rhs = qrhp.tile([64, T_TILE], F32R, tag="qr")
                        nc.sync.dma_start(qr_rhs[:], qT_rope_r[:, h, qsl])
                        av_ps = avsp.tile([128, T_TILE], F32, tag="av")
                        l_ps = lpsp.tile([1, T_TILE], F32, tag="l")
                        for kb in range(nkb):
                            sps = spsp.tile([128, T_TILE], F32, tag="s")
                            nc.tensor.matmul(
                                sps[:],
                                kn_sb[:, h, kb * 128:(kb + 1) * 128],
                                qn_rhs[:], start=True, stop=False)
                            nc.tensor.matmul(
                                sps[:], krot_sb[:, kb * 128:(kb + 1) * 128],
                                qr_rhs[:], start=False, stop=True)
                            pt = ptp.tile([128, T_TILE], F32R, tag="p")
                            nc.scalar.activation(
                                pt[:], sps[:],
                                mybir.ActivationFunctionType.Exp, scale=SCALE)
                            j = kb - 4 * qt
                            if j >= 0:
                                nc.vector.tensor_tensor(
                                    pt[:], pt[:], masks_t[:, j, :],
                                    mybir.AluOpType.mult)
                            nc.tensor.matmul(
                                av_ps[:], vh_sb[:, h, kb, :], pt[:],
                                start=(kb == 0), stop=(kb == nkb - 1))
                            nc.tensor.matmul(
                                l_ps[:], ones_t[:], pt[:],
                                start=(kb == 0), stop=(kb == nkb - 1))
                        rec = qrhp.tile([1, T_TILE], F32, tag="rec")
                        nc.vector.reciprocal(rec[:], l_ps[:])
                        rec_b = qrhp.tile([128, T_TILE], F32, tag="recb")
                        nc.gpsimd.partition_broadcast(rec_b[:], rec[:])
                        nc.vector.tensor_tensor(
                            at_full[:, h, :], av_ps[:], rec_b[:],
                            mybir.AluOpType.mult)
                        if debug:
                            nc.sync.dma_start(attn_T.rearrange(
                                "(f p) s -> p f s", p=128)[:, h, qsl],
                                at_full[:, h, :])
                    # fused o_proj for this q-tile
                    for ts in range(T_TILE // 128):
                        tok0 = qt * T_TILE + ts * 128
                        for ho in range(HID // T_TILE):
                            ps = posp.tile([128, T_TILE], F32, tag="po")
                            for fs in range(HPC):
                                nc.tensor.matmul(
                                    ps[:],
                                    at_full[:, fs, ts * 128:(ts + 1) * 128],
                                    wo_sb[:, fs,
                                          ho * T_TILE:(ho + 1) * T_TILE],
                                    start=(fs == 0), stop=(fs == HPC - 1))
                            oe = oep.tile([128, T_TILE], F32, tag="oe")
                            nc.scalar.activation(
                                oe[:], ps[:],
                                mybir.ActivationFunctionType.Copy)
                            nc.sync.dma_start(
                                out[tok0:tok0 + 128,
                                    ho * T_TILE:(ho + 1) * T_TILE],
                                oe[:])
            kv_res.release()

    nc.compile()
    _BUILD_CACHE[key] = nc
    return nc


def _host_consts():
    ivf = (1.0 / (ROPE_BASE ** (np.arange(0, DR, 2, dtype=np.float64) / DR)))
    ivf = ivf.astype(np.float32)                       # [32]
    inv_freq128 = np.tile(ivf, 4).reshape(128, 1)

    rot = np.zeros((DR, DR), np.float32)               # rot(x) = P @ x
    for d in range(32):
        rot[d, d + 32] = -1.0
        rot[d + 32, d] = 1.0
    rotT = rot.T
    p128 = np.zeros((128, 128), np.float32)
    p128[:64, :64] = rotT
    p128[64:, 64:] = rotT

    kk = np.arange(128)[None, :, None]                 # [1,128,1]
    jj = np.arange(4)[:, None, None]                   # [4,1,1]
    qq = np.arange(T_TILE)[None, None, :]              # [1,1,512]
    masks = ((jj * 128 + kk) <= qq).astype(np.float32)  # [4,128,512]

    return inv_freq128, p128, masks


LAST_RES = None


def kernel(_debug=False, **inputs):
    hidden_states = np.asarray(inputs["hidden_states"], np.float32)
    position_ids = np.asarray(inputs["position_ids"])
    W_qa = np.asarray(inputs["W_qa"], np.float32)
    b_qa = np.asarray(inputs["b_qa"], np.float32)
    w_qa_ln = np.asarray(inputs["w_qa_ln"], np.float32)
    W_qb = np.asarray(inputs["W_qb"], np.float32)
    W_kva = np.asarray(inputs["W_kva"], np.float32)
    b_kva = np.asarray(inputs["b_kva"], np.float32)
    w_kva_ln = np.asarray(inputs["w_kva_ln"], np.float32)
    W_kvb = np.asarray(inputs["W_kvb"], np.float32)
    W_o = np.asarray(inputs["W_o"], np.float32)

    nc = build_kernel(debug=_debug)

    inv_freq128, p128, masks = _host_consts()

    w_qaT = np.ascontiguousarray(W_qa.T)
    w_kvaT = np.ascontiguousarray(W_kva.T)
    W_qb_h = W_qb.reshape(NH, QD, QLR)
    W_kvb_h = W_kvb.reshape(NH, DN + DV, KVLR)
    b_qa_t = np.ascontiguousarray(b_qa.reshape(NFO_QA, 128).T)
    b_kva_p = np.zeros(640, np.float32)
    b_kva_p[:NKV] = b_kva
    b_kva_t = np.ascontiguousarray(b_kva_p.reshape(5, 128).T)
    ln_qa_t = np.ascontiguousarray(w_qa_ln.reshape(-1, 128).T)
    ln_kva_t = np.ascontiguousarray(w_kva_ln.reshape(-1, 128).T)
    ones_col = np.ones((128, 1), np.float32)

    in_maps = []
    for c in range(N_CORES):
        b = c // TPG
        g = c % TPG
        hs = list(range(g * HPC, (g + 1) * HPC))
        # q_b columns: nope blocks by head then rope blocks by head
        qb_nope = np.concatenate([W_qb_h[h, :DN, :] for h in hs], 0)
        qb_rope = np.concatenate([W_qb_h[h, DN:, :] for h in hs], 0)
        w_qbT = np.ascontiguousarray(np.concatenate([qb_nope, qb_rope], 0).T)
        w_kvb_nT = np.ascontiguousarray(
            np.concatenate([W_kvb_h[h, :DN, :] for h in hs], 0).T)
        w_kvb_vT = np.ascontiguousarray(
            np.concatenate([W_kvb_h[h, DN:, :] for h in hs], 0).T)
        w_oT = np.ascontiguousarray(
            W_o[:, g * HPC * DV:(g + 1) * HPC * DV].T)
        pos_b = position_ids[b].astype(np.int32)
        in_maps.append({
            "xTl": np.ascontiguousarray(
                hidden_states[b].T[:, g * T_TILE:(g + 1) * T_TILE]),
            "w_qaT": w_qaT, "w_kvaT": w_kvaT,
            "w_qbT": w_qbT, "w_kvb_nT": w_kvb_nT, "w_kvb_vT": w_kvb_vT,
            "w_oT": w_oT,
            "b_qa": b_qa_t, "b_kva": b_kva_t,
            "ln_qa": ln_qa_t, "ln_kva": ln_kva_t,
            "pos": np.ascontiguousarray(pos_b.reshape(1, S)),
            "pos_l": np.ascontiguousarray(
                pos_b[g * T_TILE:(g + 1) * T_TILE].reshape(1, T_TILE)),
            "inv_freq": inv_freq128,
            "p128": p128, "ones_col": ones_col,
            "masks": masks,
        })

    res = run_bass_kernel_spmd(nc, in_maps, list(range(N_CORES)))
    global LAST_RES
    LAST_RES = res

    out = np.zeros((B, S, HID), np.float32)
    for c in range(N_CORES):
        out[c // TPG] += res.results[c]["out"]
    return out


if __name__ == "__main__":
    import time
    t0 = time.time()
    build_kernel()
    print(f"build+compile: {time.time()-t0:.1f}s")



# revision 8
# speedup vs baseline: 1.3745x; 1.3745x over previous
"""DeepSeekV2-style MLA attention forward on 8 Trainium2 NeuronCores.

Sharding: 2-way data-parallel over batch x 4-way tensor-parallel over heads
(4 heads per core). The shared low-rank q_a/kv_a projections are sharded
over TOKENS within each batch's TP group: each core projects+rmsnorms its
own quarter of the sequence fully locally, then AllGathers (in bf16)
replicate them. o_proj partial outputs are summed on the host (TP unshard).

Pipeline (per core): kv_a path first -> ckv AllGather issued early ->
q_a path -> qa AllGather.  proj_b runs kv_b/v (needs only the early ckv
gather) before q_b (needs the qa gather), so PE work overlaps the second
collective.  q/k/v stay resident in SBUF in bf16 between proj_b and
attention; all proj_b/attention matmuls run in bf16 (same PE rate as f32r
at >=256-wide moving operands, half the DMA/SBUF traffic).  Phase A
matmuls stay f32r for precision.

Attention: scores are computed transposed s^T[k, q] (k on partitions),
exp'd on ScalarE without max-subtraction (scores are provably small here),
masked on the causal diagonal blocks. The softmax denominator comes from
VectorE-accumulated probs + one ones[128,128] matmul per (head, q-tile)
producing a partition-broadcast sum, inverted on ScalarE. o_proj is fused
into the same q-tile loop; output is written bf16 and upcast on host.
"""
import math
import sys

import numpy as np

try:
    import concourse.bass as bass  # noqa: F401
except ImportError:  # pragma: no cover
    sys.path.insert(0, "/opt/trn_rl_repo")

import concourse.bass as bass
import concourse.tile as tile
from concourse import bacc, mybir
from concourse.bass_utils import run_bass_kernel_spmd

import ml_dtypes

BF16NP = ml_dtypes.bfloat16

# ---- problem dims (hardcoded per contest contract) ----
B, S, HID = 2, 2048, 2048
NH = 16
DN, DR, DV = 128, 64, 128
QD = DN + DR                       # 192
QLR, KVLR = 1536, 512
EPS = 1e-6
ROPE_BASE = 10000.0
SCALE = 1.0 / math.sqrt(QD)

N_CORES = 8
TPG = 4                            # TP group size (cores per batch)
HPC = NH // TPG                    # heads per core = 4

F32 = mybir.dt.float32
F32R = mybir.dt.float32r
BF16 = mybir.dt.bfloat16
I32 = mybir.dt.int32

NKV = KVLR + DR                    # 576 kv_a rows
T_TILE = 512                       # token tile (free dim)
NT = S // T_TILE                   # 4 token tiles
KB = S // 128                      # 16 key tiles of 128

NFO_KV = KVLR // 128               # 4
NFO_QA = QLR // 128                # 12
NHI = HID // 128                   # 16
NQB = HPC * QD // 128              # 6 q_b output tiles (4 nope + 2 rope)

TWO_PI = 2.0 * math.pi
MAGIC = np.float32(1.5 * 2**23)    # round-to-nearest-int magic constant

REPLICA_GROUPS = [[0, 1, 2, 3], [4, 5, 6, 7]]


def _cody_waite_consts():
    def trunc12(x):
        return np.frombuffer(
            (np.frombuffer(np.float32(x).tobytes(), np.uint32)
             & np.uint32(0xFFFFF000)).tobytes(), np.float32)[0]
    c1 = trunc12(np.float64(TWO_PI))
    c2 = trunc12(np.float64(TWO_PI) - np.float64(c1))
    c3 = np.float32(np.float64(TWO_PI) - np.float64(c1) - np.float64(c2))
    return float(c1), float(c2), float(c3)


CW1, CW2, CW3 = _cody_waite_consts()

_BUILD_CACHE = {}


def build_kernel(debug=False):
    key = bool(debug)
    if key in _BUILD_CACHE:
        return _BUILD_CACHE[key]

    nc = bacc.Bacc("TRN2", target_bir_lowering=False, debug=False,
                   num_devices=N_CORES)

    def din(name, shape, dt=F32R):
        return nc.dram_tensor(name, list(shape), dt, kind="ExternalInput").ap()

    # ---- per-core external inputs ----
    xTl = din("xTl", [HID, T_TILE])                # hidden^T, LOCAL tokens
    w_qaT = din("w_qaT", [HID, QLR])
    w_kvaT = din("w_kvaT", [HID, NKV])
    w_qbT = din("w_qbT", [QLR, HPC * QD], BF16)    # cols: nope h0..h3 | rope
    w_kvb_nT = din("w_kvb_nT", [KVLR, HPC * DN], BF16)
    w_kvb_vT = din("w_kvb_vT", [KVLR, HPC * DV], BF16)
    w_oT = din("w_oT", [HPC * DV, HID], BF16)
    b_qa = din("b_qa", [128, NFO_QA], F32)
    b_kva = din("b_kva", [128, 5], F32)            # 576 padded to 640
    ln_qa = din("ln_qa", [128, NFO_QA], F32)
    ln_kva = din("ln_kva", [128, NFO_KV], F32)
    pos = din("pos", [1, S], I32)                  # full positions (q rope)
    pos_l = din("pos_l", [1, T_TILE], I32)         # local positions (k rope)
    inv_freq = din("inv_freq", [128, 1], F32)      # rope inv freqs, 4x tiled
    p128 = din("p128", [128, 128], BF16)           # blockdiag(rotT, rotT)
    ones128 = din("ones128", [128, 128])           # f32r ones (col-sum mm)
    masks = din("masks", [4, 128, T_TILE], BF16)   # causal diag masks

    out = nc.dram_tensor("out", [S, HID], BF16, kind="ExternalOutput").ap()

    # collective in/out tensors must stay Internal (cannot be IO)
    qa_sh = nc.dram_tensor("qa_sh", [QLR, T_TILE], BF16).ap()
    qa_all = nc.dram_tensor("qa_all", [NT, QLR, T_TILE], BF16).ap()
    ckv_sh = nc.dram_tensor("ckv_sh", [NKV, T_TILE], BF16).ap()  # norm|k_rot
    ckv_all = nc.dram_tensor("ckv_all", [NT, NKV, T_TILE], BF16).ap()

    if debug:
        def dmid(name, shape, dt):
            return nc.dram_tensor(name, list(shape), dt,
                                  kind="ExternalOutput").ap()
        dbg_qn = dmid("dbg_qn", [128, HPC, S], BF16)
        dbg_qr = dmid("dbg_qr", [64, HPC, S], BF16)
        dbg_kn = dmid("dbg_kn", [128, HPC, S], BF16)
        dbg_v = dmid("dbg_v", [KB, 128, HPC, DV], BF16)
        dbg_krot = dmid("dbg_krot", [64, S], BF16)
        dbg_attn = dmid("dbg_attn", [128, HPC, S], BF16)

    with tile.TileContext(nc) as tc:
        with tc.tile_pool(name="const", bufs=1) as constp:
            # ---- earliest DMAs: rope inputs + x on sync, consts on gpsimd
            pos_l_t = constp.tile([1, T_TILE], I32)
            nc.sync.dma_start(pos_l_t[:], pos_l[:])
            ivf_t = constp.tile([128, 1], F32)
            nc.sync.dma_start(ivf_t[:], inv_freq[:])

            xTl_r = xTl.rearrange("(hi p) s -> p hi s", p=128)
            w_qaT_r = w_qaT.rearrange("(hi p) f -> p hi f", p=128)
            w_kvaT_r = w_kvaT.rearrange("(hi p) f -> p hi f", p=128)
            qa_sh_r = qa_sh.rearrange("(f p) s -> p f s", p=128)

            bkva_t = constp.tile([128, 5], F32)
            nc.gpsimd.dma_start(bkva_t[:], b_kva[:])
            lnkva_t = constp.tile([128, NFO_KV], F32)
            nc.gpsimd.dma_start(lnkva_t[:], ln_kva[:])
            bqa_t = constp.tile([128, NFO_QA], F32)
            nc.gpsimd.dma_start(bqa_t[:], b_qa[:])
            lnqa_t = constp.tile([128, NFO_QA], F32)
            nc.gpsimd.dma_start(lnqa_t[:], ln_qa[:])
            ones_t = constp.tile([128, 128], F32R)
            nc.gpsimd.dma_start(ones_t[:], ones128[:])
            p128_t = constp.tile([128, 128], BF16)
            nc.gpsimd.dma_start(p128_t[:], p128[:])
            pos_t = constp.tile([1, S], I32)
            nc.gpsimd.dma_start(pos_t[:], pos[:])

            def rope_tables(pos_t_ap, n, cos_dst, sin_dst, rp, tag):
                """Build cos/sin [128, n] tables from int32 positions [1,n]."""
                pos_f = rp.tile([1, n], F32, name=f"pos_f_{tag}")
                nc.vector.tensor_copy(pos_f[:], pos_t_ap)
                pos_b = rp.tile([128, n], F32, name=f"pos_b_{tag}")
                nc.gpsimd.partition_broadcast(pos_b[:], pos_f[:])
                freqs = rp.tile([128, n], F32, name=f"freqs_{tag}")
                nc.vector.tensor_scalar_mul(freqs[:], pos_b[:], ivf_t[:])
                kr = rp.tile([128, n], F32, name=f"kr_{tag}")
                nc.vector.tensor_scalar(kr[:], freqs[:], 1.0 / TWO_PI,
                                        float(MAGIC), mybir.AluOpType.mult,
                                        mybir.AluOpType.add)
                nc.vector.tensor_scalar_sub(kr[:], kr[:], float(MAGIC))
                red = rp.tile([128, n], F32, name=f"red_{tag}")
                nc.vector.cody_waite_cascade(red[:], freqs[:], kr[:],
                                             CW1, CW2, CW3)
                nc.scalar.activation(sin_dst, red[:],
                                     mybir.ActivationFunctionType.Sin)
                redc = rp.tile([128, n], F32, name=f"redc_{tag}")
                nc.vector.add_range_wrap(redc[:], red[:], math.pi / 2.0,
                                         math.pi, TWO_PI)
                nc.scalar.activation(cos_dst, redc[:],
                                     mybir.ActivationFunctionType.Sin)

            # ---------- phase A: local-token kv_a / q_a + rmsnorm + k rope
            with nc.named_scope("proj_a"), \
                 tc.tile_pool(name="ap_", bufs=1) as ap_, \
                 tc.tile_pool(name="wa", bufs=2) as wap, \
                 tc.tile_pool(name="va", bufs=1) as vap, \
                 tc.tile_pool(name="nrm", bufs=3) as nrp, \
                 tc.tile_pool(name="pa", bufs=3, space="PSUM") as pap, \
                 tc.tile_pool(name="ssp", bufs=1, space="PSUM") as ssp:
                xa = ap_.tile([128, NHI, T_TILE], F32R)
                for hi in range(NHI):
                    nc.sync.dma_start(xa[:, hi, :], xTl_r[:, hi, :])

                # local rope tables for k_pe (vector work, overlaps PE)
                cos_l = ap_.tile([128, T_TILE], F32)
                sin_l = ap_.tile([128, T_TILE], F32)
                rope_tables(pos_l_t[:], T_TILE, cos_l[:], sin_l[:], ap_,
                            "loc")

                val_qa = ap_.tile([128, NFO_QA, T_TILE], F32)
                val_kv = ap_.tile([128, 5, T_TILE], F32)
                ss_qa = ssp.tile([128, T_TILE], F32, name="ss_qa")
                ss_kv = ssp.tile([128, T_TILE], F32, name="ss_kv")

                def a_proj(proj):
                    # proj 0 = kv (5 f_outs, 4 normalized), 1 = qa (12)
                    nfo = 5 if proj == 0 else NFO_QA
                    wsrc = w_kvaT_r if proj == 0 else w_qaT_r
                    bias_t = bkva_t if proj == 0 else bqa_t
                    vdst = val_kv if proj == 0 else val_qa
                    sst = ss_kv if proj == 0 else ss_qa
                    nsq = 4 if proj == 0 else NFO_QA
                    for fo in range(nfo):
                        m = 128 if not (proj == 0 and fo == 4) else 64
                        wt = wap.tile([128, NHI, 128], F32R, tag="wt")
                        if fo == 0:
                            for hc in range(4):
                                nc.gpsimd.dma_start(
                                    wt[:, hc * 4:(hc + 1) * 4, :m],
                                    wsrc[:, hc * 4:(hc + 1) * 4,
                                         fo * 128:fo * 128 + m])
                        else:
                            nc.gpsimd.dma_start(
                                wt[:, :, :m],
                                wsrc[:, :, fo * 128:fo * 128 + m])
                        ps = pap.tile([m, T_TILE], F32, tag="acc")
                        for hi in range(NHI):
                            nc.tensor.matmul(
                                ps[:], wt[:, hi, :m], xa[:, hi, :],
                                start=(hi == 0), stop=(hi == NHI - 1))
                        nc.vector.tensor_scalar_add(
                            vdst[:m, fo, :], ps[:], bias_t[:m, fo:fo + 1])
                        if fo < nsq:
                            sq = vap.tile([128, T_TILE], F32R, tag="sq")
                            nc.vector.tensor_tensor(
                                sq[:], vdst[:, fo, :], vdst[:, fo, :],
                                mybir.AluOpType.mult)
                            nc.tensor.matmul(
                                sst[:], ones_t[:], sq[:],
                                start=(fo == 0), stop=(fo == nsq - 1))

                def a_norm(proj):
                    d = KVLR if proj == 0 else QLR
                    nfo = NFO_KV if proj == 0 else NFO_QA
                    sst = ss_kv if proj == 0 else ss_qa
                    ln_t = lnkva_t if proj == 0 else lnqa_t
                    vsrc = val_kv if proj == 0 else val_qa
                    ms = vap.tile([128, T_TILE], F32, tag="ms")
                    nc.vector.tensor_scalar(
                        ms[:], sst[:], 1.0 / d, EPS,
                        mybir.AluOpType.mult, mybir.AluOpType.add)
                    rstd_b = vap.tile([128, T_TILE], F32, tag="rstd")
                    nc.scalar.activation(
                        rstd_b[:], ms[:],
                        mybir.ActivationFunctionType.Abs_reciprocal_sqrt)
                    for f in range(nfo):
                        nrm = nrp.tile([128, T_TILE], BF16, tag="nrm")
                        nc.vector.scalar_tensor_tensor(
                            nrm[:], vsrc[:, f, :], ln_t[:, f:f + 1],
                            rstd_b[:],
                            mybir.AluOpType.mult, mybir.AluOpType.mult)
                        if proj == 0:
                            nc.sync.dma_start(
                                ckv_sh[f * 128:(f + 1) * 128, :], nrm[:])
                        else:
                            nc.sync.dma_start(qa_sh_r[:, f, :], nrm[:])

                # ---- kv path first: unblocks the ckv AllGather early
                a_proj(0)
                a_norm(0)
                # k_pe rope (local tokens) -> ckv_sh rows 512..576
                kpe = vap.tile([64, T_TILE], BF16, tag="kpe")
                nc.vector.tensor_copy(kpe[:], val_kv[0:64, 4, :])
                rps = pap.tile([64, T_TILE], F32, tag="rotk")
                nc.tensor.matmul(rps[:], p128_t[0:64, 0:64], kpe[:],
                                 start=True, stop=True)
                tmp = vap.tile([64, T_TILE], F32, tag="tmpk")
                nc.vector.tensor_tensor(tmp[:], cos_l[0:64, :], kpe[:],
                                        mybir.AluOpType.mult)
                rot = vap.tile([64, T_TILE], F32, tag="rotk2")
                nc.vector.tensor_tensor(rot[:], sin_l[0:64, :], rps[:],
                                        mybir.AluOpType.mult)
                kro = vap.tile([64, T_TILE], BF16, tag="kro")
                nc.vector.tensor_tensor(kro[:], tmp[:], rot[:],
                                        mybir.AluOpType.add)
                nc.sync.dma_start(ckv_sh[KVLR:KVLR + DR, :], kro[:])
                nc.gpsimd.collective_compute(
                    "AllGather", mybir.AluOpType.bypass,
                    replica_groups=REPLICA_GROUPS,
                    ins=[ckv_sh[:]], outs=[ckv_all[:]])

                # ---- q path
                a_proj(1)
                a_norm(1)
                nc.gpsimd.collective_compute(
                    "AllGather", mybir.AluOpType.bypass,
                    replica_groups=REPLICA_GROUPS,
                    ins=[qa_sh[:]], outs=[qa_all[:]])

            # ---------- phase R: full rope cos/sin tables (for q rope) ----
            cos_t = constp.tile([128, NT, T_TILE], F32)
            sin_t = constp.tile([128, NT, T_TILE], F32)
            with nc.named_scope("rope_tables"), \
                 tc.tile_pool(name="ropep", bufs=1) as rp:
                rope_tables(pos_t[:], S, cos_t.rearrange("p n t -> p (n t)"),
                            sin_t.rearrange("p n t -> p (n t)"), rp, "full")

            # B->C resident tiles (bf16), direct-written by phase B
            kv_res = tc.alloc_tile_pool(name="kv_res", bufs=1)
            kn_sb = kv_res.tile([128, HPC, S], BF16, name="kn_sb")
            vh_sb = kv_res.tile([128, HPC, KB, DV], BF16, name="vh_sb")
            krot_sb = kv_res.tile([64, S], BF16, name="krot_sb")
            qn_sb = kv_res.tile([128, HPC, S], BF16, name="qn_sb")
            qr_sb = kv_res.tile([64, HPC, S], BF16, name="qr_sb")

            # ---------- phase B: kv_b / v first (needs ckv ring only),
            # then q_b + q rope (needs qa ring)
            w_qbT_r = w_qbT.rearrange("(fi p) f -> p fi f", p=128)
            w_kvb_nT_r = w_kvb_nT.rearrange("(fi p) f -> p fi f", p=128)
            w_kvb_vT_r = w_kvb_vT.rearrange("(fi p) f -> p fi f", p=128)
            w_oT_r = w_oT.rearrange("(fs p) hid -> p fs hid", p=128)

            cwp = tc.alloc_tile_pool(name="cw", bufs=1)
            wo_sb = cwp.tile([128, HPC, HID], BF16, name="wo_sb")
            masks_t = cwp.tile([128, 4, T_TILE], BF16, name="masks_t")

            with nc.named_scope("proj_b"), \
                 tc.tile_pool(name="wb", bufs=1) as wbp, \
                 tc.tile_pool(name="rhb", bufs=2) as rhbp, \
                 tc.tile_pool(name="evb", bufs=2) as evbp, \
                 tc.tile_pool(name="pb", bufs=2, space="PSUM") as pbp:
                wv_t = wbp.tile([128, NFO_KV, HPC * DV], BF16, name="wv_t")
                nc.gpsimd.dma_start(wv_t[:], w_kvb_vT_r[:])
                wkn_t = wbp.tile([128, NFO_KV, HPC * DN], BF16, name="wkn_t")
                nc.gpsimd.dma_start(wkn_t[:], w_kvb_nT_r[:])
                wqb_t = wbp.tile([128, NFO_QA, HPC * QD], BF16, name="wqb_t")
                nc.gpsimd.dma_start(wqb_t[:], w_qbT_r[:])
                nc.gpsimd.dma_start(masks_t[:],
                                    masks.rearrange("j p t -> p j t"))
                nc.gpsimd.dma_start(wo_sb[:], w_oT_r[:])

                # ---- kv_b nope + v for all tiles (dep: ckv AllGather)
                for t in range(NT):
                    tsl = slice(t * T_TILE, (t + 1) * T_TILE)
                    ckv_rhs = rhbp.tile([128, NFO_KV, T_TILE], BF16,
                                        tag="ckvrhs")
                    for f in range(NFO_KV):
                        nc.sync.dma_start(
                            ckv_rhs[:, f, :],
                            ckv_all[t, f * 128:(f + 1) * 128, :])
                    nc.sync.dma_start(
                        krot_sb[:, tsl], ckv_all[t, KVLR:KVLR + DR, :])
                    for fo in range(HPC):
                        ps = pbp.tile([128, T_TILE], F32, tag="pb")
                        for fi in range(NFO_KV):
                            nc.tensor.matmul(
                                ps[:], wkn_t[:, fi, fo * 128:(fo + 1) * 128],
                                ckv_rhs[:, fi, :],
                                start=(fi == 0), stop=(fi == NFO_KV - 1))
                        nc.scalar.activation(
                            kn_sb[:, fo, tsl], ps[:],
                            mybir.ActivationFunctionType.Copy)
                    for ts in range(T_TILE // 128):
                        kb = t * 4 + ts
                        ps = pbp.tile([128, HPC * DV], F32, tag="pb")
                        for fi in range(NFO_KV):
                            nc.tensor.matmul(
                                ps[:],
                                ckv_rhs[:, fi, ts * 128:(ts + 1) * 128],
                                wv_t[:, fi, :],
                                start=(fi == 0), stop=(fi == NFO_KV - 1))
                        nc.scalar.activation(
                            vh_sb[:, :, kb, :],
                            ps[:].rearrange("p (h d) -> p h d", h=HPC),
                            mybir.ActivationFunctionType.Copy)

                # ---- q_b + q rope for all tiles (dep: qa AllGather)
                for t in range(NT):
                    tsl = slice(t * T_TILE, (t + 1) * T_TILE)
                    qa_rhs = rhbp.tile([128, NFO_QA, T_TILE], BF16,
                                       tag="qarhs")
                    for f in range(NFO_QA):
                        nc.sync.dma_start(
                            qa_rhs[:, f, :],
                            qa_all[t, f * 128:(f + 1) * 128, :])
                    for fo in range(NQB):
                        ps = pbp.tile([128, T_TILE], F32, tag="pb")
                        for fi in range(NFO_QA):
                            nc.tensor.matmul(
                                ps[:], wqb_t[:, fi, fo * 128:(fo + 1) * 128],
                                qa_rhs[:, fi, :],
                                start=(fi == 0), stop=(fi == NFO_QA - 1))
                        if fo < HPC:  # nope
                            nc.scalar.activation(
                                qn_sb[:, fo, tsl], ps[:],
                                mybir.ActivationFunctionType.Copy)
                        else:  # rope pair: rows = heads (2j, 2j+1)
                            j = fo - HPC
                            qpe = evbp.tile([128, T_TILE], BF16, tag="qpe")
                            nc.scalar.activation(
                                qpe[:], ps[:],
                                mybir.ActivationFunctionType.Copy)
                            rps = pbp.tile([128, T_TILE], F32, tag="rot",
                                           bufs=1)
                            nc.tensor.matmul(rps[:], p128_t[:], qpe[:],
                                             start=True, stop=True)
                            tmp = evbp.tile([128, T_TILE], F32, tag="tmp")
                            nc.vector.tensor_tensor(
                                tmp[:], cos_t[:, t, :], qpe[:],
                                mybir.AluOpType.mult)
                            rot = evbp.tile([128, T_TILE], F32, tag="rot2")
                            nc.vector.tensor_tensor(
                                rot[:], sin_t[:, t, :], rps[:],
                                mybir.AluOpType.mult)
                            nc.vector.tensor_tensor(
                                qr_sb[:, 2 * j, tsl], tmp[0:64, :],
                                rot[0:64, :], mybir.AluOpType.add)
                            nc.vector.tensor_tensor(
                                qr_sb[:, 2 * j + 1, tsl], tmp[64:128, :],
                                rot[64:128, :], mybir.AluOpType.add)

            if debug:
                nc.sync.dma_start(dbg_kn.rearrange("p h s -> p h s")[:],
                                  kn_sb[:])
                nc.sync.dma_start(dbg_v.rearrange("kb p h d -> p h kb d"),
                                  vh_sb[:])
                nc.sync.dma_start(dbg_krot[:], krot_sb[:])
                nc.sync.dma_start(dbg_qn[:], qn_sb[:])
                nc.sync.dma_start(dbg_qr[:], qr_sb[:])

            # ---------- phase C: attention + fused o_proj ----------
            with nc.named_scope("attn"), \
                 tc.tile_pool(name="pt", bufs=4) as ptp, \
                 tc.tile_pool(name="acc", bufs=2) as accp, \
                 tc.tile_pool(name="ao", bufs=2) as aop, \
                 tc.tile_pool(name="oe", bufs=3) as oep, \
                 tc.tile_pool(name="sps", bufs=2, space="PSUM") as spsp, \
                 tc.tile_pool(name="avs", bufs=2, space="PSUM") as avsp, \
                 tc.tile_pool(name="lps", bufs=1, space="PSUM") as lpsp, \
                 tc.tile_pool(name="pos_", bufs=2, space="PSUM") as posp:
                for qt in range(NT):
                    qsl = slice(qt * T_TILE, (qt + 1) * T_TILE)
                    at_full = aop.tile([128, HPC, T_TILE], BF16, tag="atf")
                    nkb = 4 * qt + 4
                    for h in range(HPC):
                        qr_rhs = qr_sb[:, h, qsl]
                        av_ps = avsp.tile([128, T_TILE], F32, tag="av")
                        acc = accp.tile([128, T_TILE], F32R, tag="acc")
                        for kb in range(nkb):
                            sps = spsp.tile([128, T_TILE], F32, tag="s")
                            nc.tensor.matmul(
                                sps[:],
                                kn_sb[:, h, kb * 128:(kb + 1) * 128],
                                qn_sb[:, h, qsl], start=True, stop=False)
                            nc.tensor.matmul(
                                sps[:], krot_sb[:, kb * 128:(kb + 1) * 128],
                                qr_rhs, start=False, stop=True)
                            pt = ptp.tile([128, T_TILE], BF16, tag="p")
                            nc.scalar.activation(
                                pt[:], sps[:],
                                mybir.ActivationFunctionType.Exp, scale=SCALE)
                            j = kb - 4 * qt
                            if j >= 0:
                                nc.vector.tensor_tensor(
                                    pt[:], pt[:], masks_t[:, j, :],
                                    mybir.AluOpType.mult)
                            if kb == 0:
                                nc.vector.tensor_copy(acc[:], pt[:])
                            else:
                                nc.vector.tensor_tensor(
                                    acc[:], acc[:], pt[:],
                                    mybir.AluOpType.add)
                            nc.tensor.matmul(
                                av_ps[:], vh_sb[:, h, kb, :], pt[:],
                                start=(kb == 0), stop=(kb == nkb - 1))
                        l_bc = lpsp.tile([128, T_TILE], F32, tag="l")
                        nc.tensor.matmul(l_bc[:], ones_t[:], acc[:],
                                         start=True, stop=True)
                        rec = ptp.tile([128, T_TILE], F32, tag="rec")
                        nc.vector.reciprocal(rec[:], l_bc[:])
                        nc.vector.tensor_tensor(
                            at_full[:, h, :], av_ps[:], rec[:],
                            mybir.AluOpType.mult)
                        if debug:
                            nc.sync.dma_start(dbg_attn[:, h, qsl],
                                              at_full[:, h, :])
                    # fused o_proj for this q-tile
                    for ts in range(T_TILE // 128):
                        tok0 = qt * T_TILE + ts * 128
                        for ho in range(HID // T_TILE):
                            ps = posp.tile([128, T_TILE], F32, tag="po")
                            for fs in range(HPC):
                                nc.tensor.matmul(
                                    ps[:],
                                    at_full[:, fs, ts * 128:(ts + 1) * 128],
                                    wo_sb[:, fs,
                                          ho * T_TILE:(ho + 1) * T_TILE],
                                    start=(fs == 0), stop=(fs == HPC - 1))
                            oe = oep.tile([128, T_TILE], BF16, tag="oe")
                            nc.scalar.activation(
                                oe[:], ps[:],
                                mybir.ActivationFunctionType.Copy)
                            nc.sync.dma_start(
                                out[tok0:tok0 + 128,
                                    ho * T_TILE:(ho + 1) * T_TILE],
                                oe[:])
            cwp.release()
            kv_res.release()

    nc.compile()
    _BUILD_CACHE[key] = nc
    return nc


def _host_consts():
    ivf = (1.0 / (ROPE_BASE ** (np.arange(0, DR, 2, dtype=np.float64) / DR)))
    ivf = ivf.astype(np.float32)                       # [32]
    inv_freq128 = np.tile(ivf, 4).reshape(128, 1)

    rot = np.zeros((DR, DR), np.float32)               # rot(x) = P @ x
    for d in range(32):
        rot[d, d + 32] = -1.0
        rot[d + 32, d] = 1.0
    rotT = rot.T
    p128 = np.zeros((128, 128), np.float32)
    p128[:64, :64] = rotT
    p128[64:, 64:] = rotT

    kk = np.arange(128)[None, :, None]                 # [1,128,1]
    jj = np.arange(4)[:, None, None]                   # [4,1,1]
    qq = np.arange(T_TILE)[None, None, :]              # [1,1,512]
    masks = ((jj * 128 + kk) <= qq).astype(np.float32)  # [4,128,512]

    return inv_freq128, p128, masks


LAST_RES = None


def kernel(_debug=False, **inputs):
    hidden_states = np.asarray(inputs["hidden_states"], np.float32)
    position_ids = np.asarray(inputs["position_ids"])
    W_qa = np.asarray(inputs["W_qa"], np.float32)
    b_qa = np.asarray(inputs["b_qa"], np.float32)
    w_qa_ln = np.asarray(inputs["w_qa_ln"], np.float32)
    W_qb = np.asarray(inputs["W_qb"], np.float32)
    W_kva = np.asarray(inputs["W_kva"], np.float32)
    b_kva = np.asarray(inputs["b_kva"], np.float32)
    w_kva_ln = np.asarray(inputs["w_kva_ln"], np.float32)
    W_kvb = np.asarray(inputs["W_kvb"], np.float32)
    W_o = np.asarray(inputs["W_o"], np.float32)

    nc = build_kernel(debug=_debug)

    inv_freq128, p128, masks = _host_consts()

    w_qaT = np.ascontiguousarray(W_qa.T)
    w_kvaT = np.ascontiguousarray(W_kva.T)
    W_qb_h = W_qb.reshape(NH, QD, QLR)
    W_kvb_h = W_kvb.reshape(NH, DN + DV, KVLR)
    b_qa_t = np.ascontiguousarray(b_qa.reshape(NFO_QA, 128).T)
    b_kva_p = np.zeros(640, np.float32)
    b_kva_p[:NKV] = b_kva
    b_kva_t = np.ascontiguousarray(b_kva_p.reshape(5, 128).T)
    ln_qa_t = np.ascontiguousarray(w_qa_ln.reshape(-1, 128).T)
    ln_kva_t = np.ascontiguousarray(w_kva_ln.reshape(-1, 128).T)
    ones128 = np.ones((128, 128), np.float32)

    in_maps = []
    for c in range(N_CORES):
        b = c // TPG
        g = c % TPG
        hs = list(range(g * HPC, (g + 1) * HPC))
        # q_b columns: nope blocks by head then rope blocks by head
        qb_nope = np.concatenate([W_qb_h[h, :DN, :] for h in hs], 0)
        qb_rope = np.concatenate([W_qb_h[h, DN:, :] for h in hs], 0)
        w_qbT = np.ascontiguousarray(np.concatenate([qb_nope, qb_rope], 0).T)
        w_kvb_nT = np.ascontiguousarray(
            np.concatenate([W_kvb_h[h, :DN, :] for h in hs], 0).T)
        w_kvb_vT = np.ascontiguousarray(
            np.concatenate([W_kvb_h[h, DN:, :] for h in hs], 0).T)
        w_oT = np.ascontiguousarray(
            W_o[:, g * HPC * DV:(g + 1) * HPC * DV].T)
        pos_b = position_ids[b].astype(np.int32)
        in_maps.append({
            "xTl": np.ascontiguousarray(
                hidden_states[b].T[:, g * T_TILE:(g + 1) * T_TILE]),
            "w_qaT": w_qaT, "w_kvaT": w_kvaT,
            "w_qbT": w_qbT.astype(BF16NP),
            "w_kvb_nT": w_kvb_nT.astype(BF16NP),
            "w_kvb_vT": w_kvb_vT.astype(BF16NP),
            "w_oT": w_oT.astype(BF16NP),
            "b_qa": b_qa_t, "b_kva": b_kva_t,
            "ln_qa": ln_qa_t, "ln_kva": ln_kva_t,
            "pos": np.ascontiguousarray(pos_b.reshape(1, S)),
            "pos_l": np.ascontiguousarray(
                pos_b[g * T_TILE:(g + 1) * T_TILE].reshape(1, T_TILE)),
            "inv_freq": inv_freq128,
            "p128": p128.astype(BF16NP), "ones128": ones128,
            "masks": masks.astype(BF16NP),
        })

    res = run_bass_kernel_spmd(nc, in_maps, list(range(N_CORES)))
    global LAST_RES
    LAST_RES = res

    out = np.zeros((B, S, HID), np.float32)
    for c in range(N_CORES):
        out[c // TPG] += res.results[c]["out"].astype(np.float32)
    return out


if __name__ == "__main__":
    import time
    t0 = time.time()
    build_kernel()
    print(f"build+compile: {time.time()-t0:.1f}s")


# revision 18
# speedup vs baseline: 1.4305x; 1.0408x over previous
"""DeepSeekV2-style MLA attention forward on 8 Trainium2 NeuronCores.

Sharding: 2-way data-parallel over batch x 4-way tensor-parallel over heads
(4 heads per core). The shared low-rank q_a/kv_a projections are sharded
over TOKENS within each batch's TP group; bf16 AllGathers replicate them.
o_proj partial outputs are summed on the host (TP unshard).

Collective pipeline: gathers move RAW (bias-added, un-normalized) latents;
the rmsnorm weight is folded into W_qb/W_kvb on the host and the per-token
rstd travels as one extra bf16 row appended to the gathered tensors, then
is applied at proj_b eviction time (VectorE) after a K=1 ones-row matmul
broadcasts it across partitions.  This lets each gather launch as soon as
its raw chunk exists: ring order kv -> qa(lo) -> qa(hi), serialized on the
gpsimd queue, overlapped with the q_a projection and proj_b kv work.  A
dummy warmup collective at t=0 absorbs cross-core launch skew.

All proj_b/attention matmuls run in bf16 (same PE rate as f32r at >=256
wide, half the traffic); phase A matmuls stay f32r for precision. q/k/v
stay SBUF-resident between proj_b and attention.  Attention scores are
computed transposed s^T[k, q], exp'd on ScalarE without max-subtraction,
masked on causal diagonal blocks; the softmax denominator comes from
VectorE-accumulated probs + one ones[128,128] matmul per (head, q-tile),
inverted as rsqrt(l)^2. o_proj is fused per q-tile; its evictions + output
DMA run on the (post-collective idle) gpsimd queue; out is bf16, upcast
host-side.
"""
import math
import sys

import numpy as np

try:
    import concourse.bass as bass  # noqa: F401
except ImportError:  # pragma: no cover
    sys.path.insert(0, "/opt/trn_rl_repo")

import concourse.bass as bass
import concourse.tile as tile
from concourse import bacc, mybir
from concourse.bass_utils import run_bass_kernel_spmd

import ml_dtypes

BF16NP = ml_dtypes.bfloat16

# ---- problem dims (hardcoded per contest contract) ----
B, S, HID = 2, 2048, 2048
NH = 16
DN, DR, DV = 128, 64, 128
QD = DN + DR                       # 192
QLR, KVLR = 1536, 512
EPS = 1e-6
ROPE_BASE = 10000.0
SCALE = 1.0 / math.sqrt(QD)

N_CORES = 8
TPG = 4                            # TP group size (cores per batch)
HPC = NH // TPG                    # heads per core = 4

F32 = mybir.dt.float32
F32R = mybir.dt.float32r
BF16 = mybir.dt.bfloat16
I32 = mybir.dt.int32

NKV = KVLR + DR                    # 576 kv_a rows
T_TILE = 512                       # token tile (free dim)
NT = S // T_TILE                   # 4 token tiles
KB = S // 128                      # 16 key tiles of 128

NFO_KV = KVLR // 128               # 4
NFO_QA = QLR // 128                # 12
NHI = HID // 128                   # 16
NQB = HPC * QD // 128              # 6 q_b output tiles (4 nope + 2 rope)
QH = NFO_QA // 2                   # qa ring chunk: 6 f_out tiles

TWO_PI = 2.0 * math.pi
MAGIC = np.float32(1.5 * 2**23)    # round-to-nearest-int magic constant

REPLICA_GROUPS = [[0, 1, 2, 3], [4, 5, 6, 7]]


def _cody_waite_consts():
    def trunc12(x):
        return np.frombuffer(
            (np.frombuffer(np.float32(x).tobytes(), np.uint32)
             & np.uint32(0xFFFFF000)).tobytes(), np.float32)[0]
    c1 = trunc12(np.float64(TWO_PI))
    c2 = trunc12(np.float64(TWO_PI) - np.float64(c1))
    c3 = np.float32(np.float64(TWO_PI) - np.float64(c1) - np.float64(c2))
    return float(c1), float(c2), float(c3)


CW1, CW2, CW3 = _cody_waite_consts()

_BUILD_CACHE = {}


def build_kernel(debug=False):
    key = bool(debug)
    if key in _BUILD_CACHE:
        return _BUILD_CACHE[key]

    nc = bacc.Bacc("TRN2", target_bir_lowering=False, debug=False,
                   num_devices=N_CORES)

    def din(name, shape, dt=F32R):
        return nc.dram_tensor(name, list(shape), dt, kind="ExternalInput").ap()

    # ---- per-core external inputs ----
    xTl = din("xTl", [HID, T_TILE])                # hidden^T, LOCAL tokens
    w_qaT = din("w_qaT", [HID, QLR])
    w_kvaT = din("w_kvaT", [HID, NKV])
    w_qbT = din("w_qbT", [QLR, HPC * QD], BF16)    # ln-folded; nope|rope cols
    w_kvb_nT = din("w_kvb_nT", [KVLR, HPC * DN], BF16)  # ln-folded
    w_kvb_vT = din("w_kvb_vT", [KVLR, HPC * DV], BF16)  # ln-folded
    w_oT = din("w_oT", [HPC * DV, HID], BF16)
    b_qa = din("b_qa", [128, NFO_QA], F32)
    b_kva = din("b_kva", [128, 5], F32)            # 576 padded to 640
    pos = din("pos", [1, S], I32)                  # full positions (q rope)
    pos_l = din("pos_l", [1, T_TILE], I32)         # local positions (k rope)
    inv_freq = din("inv_freq", [128, 1], F32)      # rope inv freqs, 4x tiled
    p128 = din("p128", [128, 128], BF16)           # blockdiag(rotT, rotT)
    ones128 = din("ones128", [128, 128])           # f32r ones (col-sum mm)
    ones_row = din("ones_row", [1, 128])           # f32r (bcast mm lhsT)
    masks = din("masks", [4, 128, T_TILE], BF16)   # causal diag masks

    out = nc.dram_tensor("out", [S, HID], BF16, kind="ExternalOutput").ap()

    # collective in/out tensors must stay Internal (cannot be IO)
    warm_sh = nc.dram_tensor("warm_sh", [1, 128], F32R).ap()
    warm_all = nc.dram_tensor("warm_all", [4, 128], F32R).ap()
    ckv_sh = nc.dram_tensor("ckv_sh", [NKV + 1, T_TILE], BF16).ap()
    ckv_all = nc.dram_tensor("ckv_all", [NT, NKV + 1, T_TILE], BF16).ap()
    qa_sh1 = nc.dram_tensor("qa_sh1", [QH * 128, T_TILE], BF16).ap()
    qa_all1 = nc.dram_tensor("qa_all1", [NT, QH * 128, T_TILE], BF16).ap()
    qa_sh2 = nc.dram_tensor("qa_sh2", [QH * 128 + 1, T_TILE], BF16).ap()
    qa_all2 = nc.dram_tensor("qa_all2", [NT, QH * 128 + 1, T_TILE],
                             BF16).ap()

    if debug:
        def dmid(name, shape, dt):
            return nc.dram_tensor(name, list(shape), dt,
                                  kind="ExternalOutput").ap()
        dbg_qn = dmid("dbg_qn", [128, HPC, S], BF16)
        dbg_qr = dmid("dbg_qr", [64, HPC, S], BF16)
        dbg_kn = dmid("dbg_kn", [128, HPC, S], BF16)
        dbg_v = dmid("dbg_v", [KB, 128, HPC, DV], BF16)
        dbg_krot = dmid("dbg_krot", [64, S], BF16)
        dbg_attn = dmid("dbg_attn", [128, HPC, S], BF16)

    with tile.TileContext(nc) as tc:
        with tc.tile_pool(name="const", bufs=1) as constp:
            # warmup collective first: absorbs cross-core launch skew on the
            # gpsimd queue while other queues do the initial DMAs
            nc.gpsimd.collective_compute(
                "AllGather", mybir.AluOpType.bypass,
                replica_groups=REPLICA_GROUPS,
                ins=[warm_sh[:]], outs=[warm_all[:]])

            # consts on the scalar queue (tiny, early)
            pos_l_t = constp.tile([1, T_TILE], I32)
            nc.scalar.dma_start(pos_l_t[:], pos_l[:])
            pos_t = constp.tile([1, S], I32)
            nc.scalar.dma_start(pos_t[:], pos[:])
            ivf_t = constp.tile([128, 1], F32)
            nc.scalar.dma_start(ivf_t[:], inv_freq[:])
            bkva_t = constp.tile([128, 5], F32)
            nc.scalar.dma_start(bkva_t[:], b_kva[:])
            bqa_t = constp.tile([128, NFO_QA], F32)
            nc.scalar.dma_start(bqa_t[:], b_qa[:])
            ones_t = constp.tile([128, 128], F32R)
            nc.scalar.dma_start(ones_t[:], ones128[:])
            onesr_t = constp.tile([1, 128], F32R)
            nc.scalar.dma_start(onesr_t[:], ones_row[:])
            onesrb_t = constp.tile([1, 128], BF16)
            nc.vector.tensor_copy(onesrb_t[:], onesr_t[:])
            p128_t = constp.tile([128, 128], BF16)
            nc.scalar.dma_start(p128_t[:], p128[:])

            xTl_r = xTl.rearrange("(hi p) s -> p hi s", p=128)
            w_qaT_r = w_qaT.rearrange("(hi p) f -> p hi f", p=128)
            w_kvaT_r = w_kvaT.rearrange("(hi p) f -> p hi f", p=128)
            qa_sh1_r = qa_sh1.rearrange("(f p) s -> p f s", p=128)
            qa_sh2_r = qa_sh2[0:QH * 128, :].rearrange(
                "(f p) s -> p f s", p=128)

            tabp = tc.alloc_tile_pool(name="tab", bufs=1)
            cos_l = tabp.tile([128, T_TILE], F32, name="cos_l")
            sin_l = tabp.tile([128, T_TILE], F32, name="sin_l")
            cos_t = tabp.tile([128, NT, T_TILE], F32, name="cos_t")
            sin_t = tabp.tile([128, NT, T_TILE], F32, name="sin_t")

            def rope_chunk(pos_f_ap, cos_dst, sin_dst, rp, pp, tag):
                """cos/sin [128,512] from f32r positions [1,512] (PE bcast)."""
                pb = pp.tile([128, T_TILE], F32, tag="posbc", bufs=2)
                nc.tensor.matmul(pb[:], onesr_t[:], pos_f_ap,
                                 start=True, stop=True)
                freqs = rp.tile([128, T_TILE], F32, tag=f"fq_{tag}")
                nc.vector.tensor_scalar_mul(freqs[:], pb[:], ivf_t[:])
                kr = rp.tile([128, T_TILE], F32, tag=f"kr_{tag}")
                nc.vector.tensor_scalar(kr[:], freqs[:], 1.0 / TWO_PI,
                                        float(MAGIC), mybir.AluOpType.mult,
                                        mybir.AluOpType.add)
                nc.vector.tensor_scalar_sub(kr[:], kr[:], float(MAGIC))
                red = rp.tile([128, T_TILE], F32, tag=f"rd_{tag}")
                nc.vector.cody_waite_cascade(red[:], freqs[:], kr[:],
                                             CW1, CW2, CW3)
                nc.scalar.activation(sin_dst, red[:],
                                     mybir.ActivationFunctionType.Sin)
                redc = rp.tile([128, T_TILE], F32, tag=f"rc_{tag}")
                nc.vector.add_range_wrap(redc[:], red[:], math.pi / 2.0,
                                         math.pi, TWO_PI)
                nc.scalar.activation(cos_dst, redc[:],
                                     mybir.ActivationFunctionType.Sin)

            # ---------- phase A ----------
            with nc.named_scope("proj_a"), \
                 tc.tile_pool(name="ap_", bufs=1) as ap_, \
                 tc.tile_pool(name="wa", bufs=3) as wap, \
                 tc.tile_pool(name="va", bufs=1) as vap, \
                 tc.tile_pool(name="pa", bufs=3, space="PSUM") as pap, \
                 tc.tile_pool(name="ssp", bufs=1, space="PSUM") as ssp:
                xa = ap_.tile([128, NHI, T_TILE], F32R)
                for hi in range(4):
                    nc.sync.dma_start(xa[:, hi, :], xTl_r[:, hi, :])
                # kv weights fo=0 right behind the first x chunks
                wts_kv = []
                for fo in range(5):
                    m = 128 if fo < 4 else 64
                    wt = wap.tile([128, NHI, 128], F32R, tag="wt")
                    wts_kv.append(wt)
                    if fo == 0:
                        for hc in range(4):
                            nc.sync.dma_start(
                                wt[:, hc * 4:(hc + 1) * 4, :m],
                                w_kvaT_r[:, hc * 4:(hc + 1) * 4, :m])
                        for hi in range(4, NHI):
                            nc.sync.dma_start(xa[:, hi, :], xTl_r[:, hi, :])
                    else:
                        nc.sync.dma_start(
                            wt[:, :, :m],
                            w_kvaT_r[:, :, fo * 128:fo * 128 + m])

                # local rope tables (PE bcast + vector + sin)
                posl_f = ap_.tile([1, T_TILE], F32R, name="posl_f")
                nc.vector.tensor_copy(posl_f[:], pos_l_t[:])
                rope_chunk(posl_f[:], cos_l[:], sin_l[:], vap, pap, "loc")

                val_qa = ap_.tile([128, NFO_QA, T_TILE], BF16)
                val_kv = ap_.tile([128, 5, T_TILE], BF16)
                ss_qa = ssp.tile([128, T_TILE], F32, name="ss_qa")
                ss_kv = ssp.tile([128, T_TILE], F32, name="ss_kv")

                def a_proj_fo(proj, fo, wt):
                    m = 128 if not (proj == 0 and fo == 4) else 64
                    bias_t = bkva_t if proj == 0 else bqa_t
                    vdst = val_kv if proj == 0 else val_qa
                    sst = ss_kv if proj == 0 else ss_qa
                    nsq = 4 if proj == 0 else NFO_QA
                    ps = pap.tile([m, T_TILE], F32, tag="acc")
                    for hi in range(NHI):
                        nc.tensor.matmul(
                            ps[:], wt[:, hi, :m], xa[:, hi, :],
                            start=(hi == 0), stop=(hi == NHI - 1))
                    nc.vector.tensor_scalar_add(
                        vdst[:m, fo, :], ps[:], bias_t[:m, fo:fo + 1])
                    if fo < nsq:
                        sq = vap.tile([128, T_TILE], F32R, tag="sq")
                        nc.vector.tensor_tensor(
                            sq[:], vdst[:, fo, :], vdst[:, fo, :],
                            mybir.AluOpType.mult)
                        nc.tensor.matmul(
                            sst[:], ones_t[:], sq[:],
                            start=(fo == 0), stop=(fo == nsq - 1))

                def a_rstd(proj, dst_row):
                    d = KVLR if proj == 0 else QLR
                    sst = ss_kv if proj == 0 else ss_qa
                    ms = vap.tile([128, T_TILE], F32, tag="ms")
                    nc.vector.tensor_scalar(
                        ms[:], sst[:], 1.0 / d, EPS,
                        mybir.AluOpType.mult, mybir.AluOpType.add)
                    rrow = vap.tile([1, T_TILE], BF16, tag="rrow")
                    nc.scalar.activation(
                        rrow[:], ms[0:1, :],
                        mybir.ActivationFunctionType.Abs_reciprocal_sqrt)
                    nc.sync.dma_start(dst_row, rrow[:])

                # ---- kv path first: unblocks the ckv AllGather early
                for fo in range(5):
                    a_proj_fo(0, fo, wts_kv[fo])
                    if fo < 4:
                        nc.sync.dma_start(
                            ckv_sh[fo * 128:(fo + 1) * 128, :],
                            val_kv[:, fo, :])
                a_rstd(0, ckv_sh[NKV:NKV + 1, :])
                # k_pe rope (local tokens, raw) -> ckv_sh rows 512..576
                kpe = val_kv[0:64, 4, :]
                rps = pap.tile([64, T_TILE], F32, tag="rotk", bufs=1)
                nc.tensor.matmul(rps[:], p128_t[0:64, 0:64], kpe,
                                 start=True, stop=True)
                tmp = vap.tile([64, T_TILE], F32, tag="tmpk")
                nc.vector.tensor_tensor(tmp[:], cos_l[0:64, :], kpe,
                                        mybir.AluOpType.mult)
                rot = vap.tile([64, T_TILE], F32, tag="rotk2")
                nc.vector.tensor_tensor(rot[:], sin_l[0:64, :], rps[:],
                                        mybir.AluOpType.mult)
                kro = vap.tile([64, T_TILE], BF16, tag="kro")
                nc.vector.tensor_tensor(kro[:], tmp[:], rot[:],
                                        mybir.AluOpType.add)
                nc.sync.dma_start(ckv_sh[KVLR:KVLR + DR, :], kro[:])
                nc.gpsimd.collective_compute(
                    "AllGather", mybir.AluOpType.bypass,
                    replica_groups=REPLICA_GROUPS,
                    ins=[ckv_sh[:]], outs=[ckv_all[:]])

                # full rope tables (needed by proj_b q-rope)
                posf_f = ap_.tile([1, S], F32R, name="posf_f")
                nc.vector.tensor_copy(posf_f[:], pos_t[:])
                for t in range(NT):
                    tsl = slice(t * T_TILE, (t + 1) * T_TILE)
                    rope_chunk(posf_f[:, tsl], cos_t[:, t, :],
                               sin_t[:, t, :], vap, pap, f"f{t}")

                # ---- q path, in two ring-halves
                for half in range(2):
                    for fo in range(half * QH, (half + 1) * QH):
                        wt = wap.tile([128, NHI, 128], F32R, tag="wt")
                        nc.sync.dma_start(
                            wt[:], w_qaT_r[:, :, fo * 128:(fo + 1) * 128])
                        a_proj_fo(1, fo, wt)
                        shr = qa_sh1_r if half == 0 else qa_sh2_r
                        nc.sync.dma_start(shr[:, fo - half * QH, :],
                                          val_qa[:, fo, :])
                    if half == 0:
                        nc.gpsimd.collective_compute(
                            "AllGather", mybir.AluOpType.bypass,
                            replica_groups=REPLICA_GROUPS,
                            ins=[qa_sh1[:]], outs=[qa_all1[:]])
                    else:
                        a_rstd(1, qa_sh2[QH * 128:QH * 128 + 1, :])
                        nc.gpsimd.collective_compute(
                            "AllGather", mybir.AluOpType.bypass,
                            replica_groups=REPLICA_GROUPS,
                            ins=[qa_sh2[:]], outs=[qa_all2[:]])

            # B->C resident tiles (bf16), direct-written by phase B
            kv_res = tc.alloc_tile_pool(name="kv_res", bufs=1)
            kn_sb = kv_res.tile([128, HPC, S], BF16, name="kn_sb")
            vh_sb = kv_res.tile([128, HPC, KB, DV], BF16, name="vh_sb")
            krot_sb = kv_res.tile([64, S], BF16, name="krot_sb")
            qn_sb = kv_res.tile([128, HPC, S], BF16, name="qn_sb")
            qr_sb = kv_res.tile([64, HPC, S], BF16, name="qr_sb")
            rkv_sb = kv_res.tile([128, NT, T_TILE], BF16, name="rkv_sb")
            rqa_sb = kv_res.tile([128, NT, T_TILE], BF16, name="rqa_sb")
            rkvc_raw = kv_res.tile([128, NT, 4], BF16, name="rkvc_raw")
            rkvc_sb = kv_res.tile([128, NT, 4], F32, name="rkvc_sb")

            w_qbT_r = w_qbT.rearrange("(fi p) f -> p fi f", p=128)
            w_kvb_nT_r = w_kvb_nT.rearrange("(fi p) f -> p fi f", p=128)
            w_kvb_vT_r = w_kvb_vT.rearrange("(fi p) f -> p fi f", p=128)
            w_oT_r = w_oT.rearrange("(fs p) hid -> p fs hid", p=128)

            cwp = tc.alloc_tile_pool(name="cw", bufs=1)
            wo_sb = cwp.tile([128, HPC, HID], BF16, name="wo_sb")
            masks_t = cwp.tile([128, 4, T_TILE], BF16, name="masks_t")

            # ---------- phase B ----------
            with nc.named_scope("proj_b"), \
                 tc.tile_pool(name="wb", bufs=1) as wbp, \
                 tc.tile_pool(name="rhb", bufs=2) as rhbp, \
                 tc.tile_pool(name="evb", bufs=2) as evbp, \
                 tc.tile_pool(name="pb", bufs=2, space="PSUM") as pbp, \
                 tc.tile_pool(name="bc", bufs=2, space="PSUM") as bcp:
                wv_t = wbp.tile([128, NFO_KV, HPC * DV], BF16, name="wv_t")
                nc.sync.dma_start(wv_t[:], w_kvb_vT_r[:])
                wkn_t = wbp.tile([128, NFO_KV, HPC * DN], BF16, name="wkn_t")
                nc.sync.dma_start(wkn_t[:], w_kvb_nT_r[:])
                wqb_t = wbp.tile([128, NFO_QA, HPC * QD], BF16, name="wqb_t")
                nc.sync.dma_start(wqb_t[:], w_qbT_r[:])
                nc.sync.dma_start(masks_t[:],
                                  masks.rearrange("j p t -> p j t"))
                nc.sync.dma_start(wo_sb[:], w_oT_r[:])

                # ---- kv_b nope + v for all tiles (dep: ckv AllGather)
                for t in range(NT):
                    tsl = slice(t * T_TILE, (t + 1) * T_TILE)
                    ckv_rhs = rhbp.tile([128, NFO_KV, T_TILE], BF16,
                                        tag="ckvrhs")
                    for f in range(NFO_KV):
                        nc.scalar.dma_start(
                            ckv_rhs[:, f, :],
                            ckv_all[t, f * 128:(f + 1) * 128, :])
                    nc.scalar.dma_start(
                        krot_sb[:, tsl], ckv_all[t, KVLR:KVLR + DR, :])
                    # rstd_kv row -> bcast [128,T] and column [128,4] forms
                    rrow = rhbp.tile([1, T_TILE], BF16, tag="rrow")
                    nc.scalar.dma_start(rrow[:], ckv_all[t, NKV:NKV + 1, :])
                    nc.scalar.dma_start(
                        rkvc_raw[:, t, :],
                        ckv_all[t, NKV, :].rearrange("(ts p) -> p ts", p=128))
                    nc.vector.tensor_copy(rkvc_sb[:, t, :], rkvc_raw[:, t, :])
                    rbc = bcp.tile([128, T_TILE], F32, tag="rbc")
                    nc.tensor.matmul(rbc[:], onesrb_t[:], rrow[:],
                                     start=True, stop=True)
                    nc.vector.tensor_copy(rkv_sb[:, t, :], rbc[:])
                    for fo in range(HPC):
                        ps = pbp.tile([128, T_TILE], F32, tag="pb")
                        for fi in range(NFO_KV):
                            nc.tensor.matmul(
                                ps[:], wkn_t[:, fi, fo * 128:(fo + 1) * 128],
                                ckv_rhs[:, fi, :],
                                start=(fi == 0), stop=(fi == NFO_KV - 1))
                        nc.vector.tensor_tensor(
                            kn_sb[:, fo, tsl], ps[:], rkv_sb[:, t, :],
                            mybir.AluOpType.mult)
                    for ts in range(T_TILE // 128):
                        kb = t * 4 + ts
                        ps = pbp.tile([128, HPC * DV], F32, tag="pb")
                        for fi in range(NFO_KV):
                            nc.tensor.matmul(
                                ps[:],
                                ckv_rhs[:, fi, ts * 128:(ts + 1) * 128],
                                wv_t[:, fi, :],
                                start=(fi == 0), stop=(fi == NFO_KV - 1))
                        nc.vector.tensor_scalar_mul(
                            vh_sb[:, :, kb, :],
                            ps[:].rearrange("p (h d) -> p h d", h=HPC),
                            rkvc_sb[:, t, ts:ts + 1])

                # ---- q_b + q rope for all tiles (dep: qa AllGathers)
                for t in range(NT):
                    tsl = slice(t * T_TILE, (t + 1) * T_TILE)
                    qa_rhs = rhbp.tile([128, NFO_QA, T_TILE], BF16,
                                       tag="qarhs")
                    for f in range(NFO_QA):
                        src = qa_all1 if f < QH else qa_all2
                        nc.scalar.dma_start(
                            qa_rhs[:, f, :],
                            src[t, (f % QH) * 128:(f % QH + 1) * 128, :])
                    rrow = rhbp.tile([1, T_TILE], BF16, tag="rrow")
                    nc.scalar.dma_start(
                        rrow[:], qa_all2[t, QH * 128:QH * 128 + 1, :])
                    rbc = bcp.tile([128, T_TILE], F32, tag="rbc")
                    nc.tensor.matmul(rbc[:], onesrb_t[:], rrow[:],
                                     start=True, stop=True)
                    nc.vector.tensor_copy(rqa_sb[:, t, :], rbc[:])
                    for fo in range(NQB):
                        ps = pbp.tile([128, T_TILE], F32, tag="pb")
                        for fi in range(NFO_QA):
                            nc.tensor.matmul(
                                ps[:], wqb_t[:, fi, fo * 128:(fo + 1) * 128],
                                qa_rhs[:, fi, :],
                                start=(fi == 0), stop=(fi == NFO_QA - 1))
                        if fo < HPC:  # nope
                            nc.vector.tensor_tensor(
                                qn_sb[:, fo, tsl], ps[:], rqa_sb[:, t, :],
                                mybir.AluOpType.mult)
                        else:  # rope pair: rows = heads (2j, 2j+1)
                            j = fo - HPC
                            qpe = evbp.tile([128, T_TILE], BF16, tag="qpe")
                            nc.vector.tensor_tensor(
                                qpe[:], ps[:], rqa_sb[:, t, :],
                                mybir.AluOpType.mult)
                            rps = pbp.tile([128, T_TILE], F32, tag="rot",
                                           bufs=1)
                            nc.tensor.matmul(rps[:], p128_t[:], qpe[:],
                                             start=True, stop=True)
                            tmp = evbp.tile([128, T_TILE], F32, tag="tmp")
                            nc.vector.tensor_tensor(
                                tmp[:], cos_t[:, t, :], qpe[:],
                                mybir.AluOpType.mult)
                            rot = evbp.tile([128, T_TILE], F32, tag="rot2")
                            nc.vector.tensor_tensor(
                                rot[:], sin_t[:, t, :], rps[:],
                                mybir.AluOpType.mult)
                            nc.vector.tensor_tensor(
                                qr_sb[:, 2 * j, tsl], tmp[0:64, :],
                                rot[0:64, :], mybir.AluOpType.add)
                            nc.vector.tensor_tensor(
                                qr_sb[:, 2 * j + 1, tsl], tmp[64:128, :],
                                rot[64:128, :], mybir.AluOpType.add)

            if debug:
                nc.sync.dma_start(dbg_kn[:], kn_sb[:])
                nc.sync.dma_start(dbg_v.rearrange("kb p h d -> p h kb d"),
                                  vh_sb[:])
                nc.sync.dma_start(dbg_krot[:], krot_sb[:])
                nc.sync.dma_start(dbg_qn[:], qn_sb[:])
                nc.sync.dma_start(dbg_qr[:], qr_sb[:])

            # ---------- phase C: attention + fused o_proj ----------
            with nc.named_scope("attn"), \
                 tc.tile_pool(name="pt", bufs=4) as ptp, \
                 tc.tile_pool(name="acc", bufs=2) as accp, \
                 tc.tile_pool(name="ao", bufs=2) as aop, \
                 tc.tile_pool(name="oe", bufs=3) as oep, \
                 tc.tile_pool(name="sps", bufs=2, space="PSUM") as spsp, \
                 tc.tile_pool(name="avs", bufs=2, space="PSUM") as avsp, \
                 tc.tile_pool(name="lps", bufs=1, space="PSUM") as lpsp, \
                 tc.tile_pool(name="pos_", bufs=2, space="PSUM") as posp:
                for qt in range(NT):
                    qsl = slice(qt * T_TILE, (qt + 1) * T_TILE)
                    at_full = aop.tile([128, HPC, T_TILE], BF16, tag="atf")
                    nkb = 4 * qt + 4
                    for h in range(HPC):
                        av_ps = avsp.tile([128, T_TILE], F32, tag="av")
                        acc = accp.tile([128, T_TILE], F32R, tag="acc")
                        for kb in range(nkb):
                            sps = spsp.tile([128, T_TILE], F32, tag="s")
                            nc.tensor.matmul(
                                sps[:],
                                kn_sb[:, h, kb * 128:(kb + 1) * 128],
                                qn_sb[:, h, qsl], start=True, stop=False)
                            nc.tensor.matmul(
                                sps[:], krot_sb[:, kb * 128:(kb + 1) * 128],
                                qr_sb[:, h, qsl], start=False, stop=True)
                            pt = ptp.tile([128, T_TILE], BF16, tag="p")
                            nc.scalar.activation(
                                pt[:], sps[:],
                                mybir.ActivationFunctionType.Exp, scale=SCALE)
                            j = kb - 4 * qt
                            if j >= 0:
                                nc.vector.tensor_tensor(
                                    pt[:], pt[:], masks_t[:, j, :],
                                    mybir.AluOpType.mult)
                            if kb == 0:
                                nc.vector.tensor_copy(acc[:], pt[:])
                            else:
                                nc.vector.tensor_tensor(
                                    acc[:], acc[:], pt[:],
                                    mybir.AluOpType.add)
                            nc.tensor.matmul(
                                av_ps[:], vh_sb[:, h, kb, :], pt[:],
                                start=(kb == 0), stop=(kb == nkb - 1))
                        l_bc = lpsp.tile([128, T_TILE], F32, tag="l")
                        nc.tensor.matmul(l_bc[:], ones_t[:], acc[:],
                                         start=True, stop=True)
                        rsq = ptp.tile([128, T_TILE], F32, tag="rsq")
                        nc.scalar.activation(
                            rsq[:], l_bc[:],
                            mybir.ActivationFunctionType.Abs_reciprocal_sqrt)
                        rec = ptp.tile([128, T_TILE], F32, tag="rec")
                        nc.vector.tensor_tensor(rec[:], rsq[:], rsq[:],
                                                mybir.AluOpType.mult)
                        nc.vector.tensor_tensor(
                            at_full[:, h, :], av_ps[:], rec[:],
                            mybir.AluOpType.mult)
                        if debug:
                            nc.sync.dma_start(dbg_attn[:, h, qsl],
                                              at_full[:, h, :])
                    # fused o_proj for this q-tile (evict+out on gpsimd)
                    for ts in range(T_TILE // 128):
                        tok0 = qt * T_TILE + ts * 128
                        for ho in range(HID // T_TILE):
                            ps = posp.tile([128, T_TILE], F32, tag="po")
                            for fs in range(HPC):
                                nc.tensor.matmul(
                                    ps[:],
                                    at_full[:, fs, ts * 128:(ts + 1) * 128],
                                    wo_sb[:, fs,
                                          ho * T_TILE:(ho + 1) * T_TILE],
                                    start=(fs == 0), stop=(fs == HPC - 1))
                            oe = oep.tile([128, T_TILE], BF16, tag="oe")
                            nc.scalar.activation(
                                oe[:], ps[:],
                                mybir.ActivationFunctionType.Copy)
                            nc.gpsimd.dma_start(
                                out[tok0:tok0 + 128,
                                    ho * T_TILE:(ho + 1) * T_TILE],
                                oe[:])
            cwp.release()
            kv_res.release()
            tabp.release()

    nc.compile()
    _BUILD_CACHE[key] = nc
    return nc


def _host_consts():
    ivf = (1.0 / (ROPE_BASE ** (np.arange(0, DR, 2, dtype=np.float64) / DR)))
    ivf = ivf.astype(np.float32)                       # [32]
    inv_freq128 = np.tile(ivf, 4).reshape(128, 1)

    rot = np.zeros((DR, DR), np.float32)               # rot(x) = P @ x
    for d in range(32):
        rot[d, d + 32] = -1.0
        rot[d + 32, d] = 1.0
    rotT = rot.T
    p128 = np.zeros((128, 128), np.float32)
    p128[:64, :64] = rotT
    p128[64:, 64:] = rotT

    kk = np.arange(128)[None, :, None]                 # [1,128,1]
    jj = np.arange(4)[:, None, None]                   # [4,1,1]
    qq = np.arange(T_TILE)[None, None, :]              # [1,1,512]
    masks = ((jj * 128 + kk) <= qq).astype(np.float32)  # [4,128,512]

    return inv_freq128, p128, masks


LAST_RES = None


def kernel(_debug=False, **inputs):
    hidden_states = np.asarray(inputs["hidden_states"], np.float32)
    position_ids = np.asarray(inputs["position_ids"])
    W_qa = np.asarray(inputs["W_qa"], np.float32)
    b_qa = np.asarray(inputs["b_qa"], np.float32)
    w_qa_ln = np.asarray(inputs["w_qa_ln"], np.float32)
    W_qb = np.asarray(inputs["W_qb"], np.float32)
    W_kva = np.asarray(inputs["W_kva"], np.float32)
    b_kva = np.asarray(inputs["b_kva"], np.float32)
    w_kva_ln = np.asarray(inputs["w_kva_ln"], np.float32)
    W_kvb = np.asarray(inputs["W_kvb"], np.float32)
    W_o = np.asarray(inputs["W_o"], np.float32)

    nc = build_kernel(debug=_debug)

    inv_freq128, p128, masks = _host_consts()

    w_qaT = np.ascontiguousarray(W_qa.T)
    w_kvaT = np.ascontiguousarray(W_kva.T)
    # fold the rmsnorm scale into the B-projections
    W_qb_f = W_qb * w_qa_ln[None, :]
    W_kvb_f = W_kvb * w_kva_ln[None, :]
    W_qb_h = W_qb_f.reshape(NH, QD, QLR)
    W_kvb_h = W_kvb_f.reshape(NH, DN + DV, KVLR)
    b_qa_t = np.ascontiguousarray(b_qa.reshape(NFO_QA, 128).T)
    b_kva_p = np.zeros(640, np.float32)
    b_kva_p[:NKV] = b_kva
    b_kva_t = np.ascontiguousarray(b_kva_p.reshape(5, 128).T)
    ones128 = np.ones((128, 128), np.float32)
    ones_row = np.ones((1, 128), np.float32)

    in_maps = []
    for c in range(N_CORES):
        b = c // TPG
        g = c % TPG
        hs = list(range(g * HPC, (g + 1) * HPC))
        # q_b columns: nope blocks by head then rope blocks by head
        qb_nope = np.concatenate([W_qb_h[h, :DN, :] for h in hs], 0)
        qb_rope = np.concatenate([W_qb_h[h, DN:, :] for h in hs], 0)
        w_qbT = np.ascontiguousarray(np.concatenate([qb_nope, qb_rope], 0).T)
        w_kvb_nT = np.ascontiguousarray(
            np.concatenate([W_kvb_h[h, :DN, :] for h in hs], 0).T)
        w_kvb_vT = np.ascontiguousarray(
            np.concatenate([W_kvb_h[h, DN:, :] for h in hs], 0).T)
        w_oT = np.ascontiguousarray(
            W_o[:, g * HPC * DV:(g + 1) * HPC * DV].T)
        pos_b = position_ids[b].astype(np.int32)
        in_maps.append({
            "xTl": np.ascontiguousarray(
                hidden_states[b].T[:, g * T_TILE:(g + 1) * T_TILE]),
            "w_qaT": w_qaT, "w_kvaT": w_kvaT,
            "w_qbT": w_qbT.astype(BF16NP),
            "w_kvb_nT": w_kvb_nT.astype(BF16NP),
            "w_kvb_vT": w_kvb_vT.astype(BF16NP),
            "w_oT": w_oT.astype(BF16NP),
            "b_qa": b_qa_t, "b_kva": b_kva_t,
            "pos": np.ascontiguousarray(pos_b.reshape(1, S)),
            "pos_l": np.ascontiguousarray(
                pos_b[g * T_TILE:(g + 1) * T_TILE].reshape(1, T_TILE)),
            "inv_freq": inv_freq128,
            "p128": p128.astype(BF16NP),
            "ones128": ones128, "ones_row": ones_row,
            "masks": masks.astype(BF16NP),
        })

    res = run_bass_kernel_spmd(nc, in_maps, list(range(N_CORES)))
    global LAST_RES
    LAST_RES = res

    out = np.zeros((B, S, HID), np.float32)
    for c in range(N_CORES):
        out[c // TPG] += res.results[c]["out"].astype(np.float32)
    return out


if __name__ == "__main__":
    import time
    t0 = time.time()
    build_kernel()
    print(f"build+compile: {time.time()-t0:.1f}s")


# revision 24
# speedup vs baseline: 1.4719x; 1.0289x over previous
"""DeepSeekV2-style MLA attention forward on 8 Trainium2 NeuronCores.

Sharding: 2-way data-parallel over batch x 4-way tensor-parallel over heads
(4 heads per core). The shared low-rank q_a/kv_a projections are sharded
over TOKENS within each batch's TP group; bf16 AllGathers replicate them.
o_proj partial outputs are summed on the host (TP unshard).

Collective pipeline: gathers move RAW (bias-added, un-normalized) latents;
the rmsnorm weight is folded into W_qb/W_kvb on the host and the per-token
rstd travels as one extra bf16 row appended to the gathered tensors, then
is applied at proj_b eviction time (VectorE) after a K=1 ones-row matmul
broadcasts it across partitions.  This lets each gather launch as soon as
its raw chunk exists: ring order kv -> qa(lo) -> qa(hi), serialized on the
gpsimd queue, overlapped with the q_a projection and proj_b kv work.  A
dummy warmup collective at t=0 absorbs cross-core launch skew.

All proj_b/attention matmuls run in bf16 (same PE rate as f32r at >=256
wide, half the traffic); phase A matmuls stay f32r for precision. q/k/v
stay SBUF-resident between proj_b and attention.  Attention scores are
computed transposed s^T[k, q], exp'd on ScalarE without max-subtraction,
masked on causal diagonal blocks; the softmax denominator comes from
VectorE-accumulated probs + one ones[128,128] matmul per (head, q-tile),
inverted as rsqrt(l)^2. o_proj is fused per q-tile; its evictions + output
DMA run on the (post-collective idle) gpsimd queue; out is bf16, upcast
host-side.
"""
import math
import sys

import numpy as np

try:
    import concourse.bass as bass  # noqa: F401
except ImportError:  # pragma: no cover
    sys.path.insert(0, "/opt/trn_rl_repo")

import concourse.bass as bass
import concourse.tile as tile
from concourse import bacc, mybir
from concourse.bass_utils import run_bass_kernel_spmd

import ml_dtypes

BF16NP = ml_dtypes.bfloat16

# ---- problem dims (hardcoded per contest contract) ----
B, S, HID = 2, 2048, 2048
NH = 16
DN, DR, DV = 128, 64, 128
QD = DN + DR                       # 192
QLR, KVLR = 1536, 512
EPS = 1e-6
ROPE_BASE = 10000.0
SCALE = 1.0 / math.sqrt(QD)

N_CORES = 8
TPG = 4                            # TP group size (cores per batch)
HPC = NH // TPG                    # heads per core = 4

F32 = mybir.dt.float32
F32R = mybir.dt.float32r
BF16 = mybir.dt.bfloat16
I32 = mybir.dt.int32

NKV = KVLR + DR                    # 576 kv_a rows
T_TILE = 512                       # token tile (free dim)
NT = S // T_TILE                   # 4 token tiles
KB = S // 128                      # 16 key tiles of 128

NFO_KV = KVLR // 128               # 4
NFO_QA = QLR // 128                # 12
NHI = HID // 128                   # 16
NQB = HPC * QD // 128              # 6 q_b output tiles (4 nope + 2 rope)
QH = NFO_QA // 2                   # qa ring chunk: 6 f_out tiles

TWO_PI = 2.0 * math.pi
MAGIC = np.float32(1.5 * 2**23)    # round-to-nearest-int magic constant

REPLICA_GROUPS = [[0, 1, 2, 3], [4, 5, 6, 7]]


def _cody_waite_consts():
    def trunc12(x):
        return np.frombuffer(
            (np.frombuffer(np.float32(x).tobytes(), np.uint32)
             & np.uint32(0xFFFFF000)).tobytes(), np.float32)[0]
    c1 = trunc12(np.float64(TWO_PI))
    c2 = trunc12(np.float64(TWO_PI) - np.float64(c1))
    c3 = np.float32(np.float64(TWO_PI) - np.float64(c1) - np.float64(c2))
    return float(c1), float(c2), float(c3)


CW1, CW2, CW3 = _cody_waite_consts()

_BUILD_CACHE = {}


def build_kernel(debug=False):
    key = bool(debug)
    if key in _BUILD_CACHE:
        return _BUILD_CACHE[key]

    nc = bacc.Bacc("TRN2", target_bir_lowering=False, debug=False,
                   num_devices=N_CORES)

    def din(name, shape, dt=F32R):
        return nc.dram_tensor(name, list(shape), dt, kind="ExternalInput").ap()

    # ---- per-core external inputs ----
    xTl = din("xTl", [HID, T_TILE], BF16)          # hidden^T, LOCAL tokens
    w_qaT = din("w_qaT", [HID, QLR], BF16)
    w_kvaT = din("w_kvaT", [HID, NKV], BF16)
    w_qbT = din("w_qbT", [QLR, HPC * QD], BF16)    # ln-folded; nope|rope cols
    w_kvb_nT = din("w_kvb_nT", [KVLR, HPC * DN], BF16)  # ln-folded
    w_kvb_vT = din("w_kvb_vT", [KVLR, HPC * DV], BF16)  # ln-folded
    w_oT = din("w_oT", [HPC * DV, HID], BF16)
    b_qa = din("b_qa", [128, NFO_QA], F32)
    b_kva = din("b_kva", [128, 5], F32)            # 576 padded to 640
    pos = din("pos", [1, S], I32)                  # full positions (q rope)
    pos_l = din("pos_l", [1, T_TILE], I32)         # local positions (k rope)
    inv_freq = din("inv_freq", [128, 1], F32)      # rope inv freqs, 4x tiled
    p128 = din("p128", [128, 128], BF16)           # blockdiag(rotT, rotT)
    ones128 = din("ones128", [128, 128])           # f32r ones (col-sum mm)
    ones_row = din("ones_row", [1, 128])           # f32r (bcast mm lhsT)
    masks = din("masks", [4, 128, T_TILE], BF16)   # causal diag masks

    out = nc.dram_tensor("out", [S, HID], BF16, kind="ExternalOutput").ap()

    # collective in/out tensors must stay Internal (cannot be IO)
    warm_sh = nc.dram_tensor("warm_sh", [1, 128], F32R).ap()
    warm_all = nc.dram_tensor("warm_all", [4, 128], F32R).ap()
    ckv_sh = nc.dram_tensor("ckv_sh", [NKV + 1, T_TILE], BF16).ap()
    ckv_all = nc.dram_tensor("ckv_all", [NT, NKV + 1, T_TILE], BF16).ap()
    qa_sh1 = nc.dram_tensor("qa_sh1", [QH * 128, T_TILE], BF16).ap()
    qa_all1 = nc.dram_tensor("qa_all1", [NT, QH * 128, T_TILE], BF16).ap()
    qa_sh2 = nc.dram_tensor("qa_sh2", [QH * 128 + 1, T_TILE], BF16).ap()
    qa_all2 = nc.dram_tensor("qa_all2", [NT, QH * 128 + 1, T_TILE],
                             BF16).ap()

    if debug:
        def dmid(name, shape, dt):
            return nc.dram_tensor(name, list(shape), dt,
                                  kind="ExternalOutput").ap()
        dbg_qn = dmid("dbg_qn", [128, HPC, S], BF16)
        dbg_qr = dmid("dbg_qr", [64, HPC, S], BF16)
        dbg_kn = dmid("dbg_kn", [128, HPC, S], BF16)
        dbg_v = dmid("dbg_v", [KB, 128, HPC, DV], BF16)
        dbg_krot = dmid("dbg_krot", [64, S], BF16)
        dbg_attn = dmid("dbg_attn", [128, HPC, S], BF16)

    with tile.TileContext(nc) as tc:
        with tc.tile_pool(name="const", bufs=1) as constp:
            # warmup collective first: absorbs cross-core launch skew on the
            # gpsimd queue while other queues do the initial DMAs
            nc.gpsimd.collective_compute(
                "AllGather", mybir.AluOpType.bypass,
                replica_groups=REPLICA_GROUPS,
                ins=[warm_sh[:]], outs=[warm_all[:]])

            # consts on the scalar queue (tiny, early)
            pos_l_t = constp.tile([1, T_TILE], I32)
            nc.scalar.dma_start(pos_l_t[:], pos_l[:])
            pos_t = constp.tile([1, S], I32)
            nc.scalar.dma_start(pos_t[:], pos[:])
            ivf_t = constp.tile([128, 1], F32)
            nc.scalar.dma_start(ivf_t[:], inv_freq[:])
            bkva_t = constp.tile([128, 5], F32)
            nc.scalar.dma_start(bkva_t[:], b_kva[:])
            bqa_t = constp.tile([128, NFO_QA], F32)
            nc.scalar.dma_start(bqa_t[:], b_qa[:])
            ones_t = constp.tile([128, 128], F32R)
            nc.scalar.dma_start(ones_t[:], ones128[:])
            onesr_t = constp.tile([1, 128], F32R)
            nc.scalar.dma_start(onesr_t[:], ones_row[:])
            onesrb_t = constp.tile([1, 128], BF16)
            nc.vector.tensor_copy(onesrb_t[:], onesr_t[:])
            p128_t = constp.tile([128, 128], BF16)
            nc.scalar.dma_start(p128_t[:], p128[:])

            xTl_r = xTl.rearrange("(hi p) s -> p hi s", p=128)
            w_qaT_r = w_qaT.rearrange("(hi p) f -> p hi f", p=128)
            w_kvaT_r = w_kvaT.rearrange("(hi p) f -> p hi f", p=128)
            qa_sh1_r = qa_sh1.rearrange("(f p) s -> p f s", p=128)
            qa_sh2_r = qa_sh2[0:QH * 128, :].rearrange(
                "(f p) s -> p f s", p=128)

            tabp = tc.alloc_tile_pool(name="tab", bufs=1)
            cos_l = tabp.tile([128, T_TILE], F32, name="cos_l")
            sin_l = tabp.tile([128, T_TILE], F32, name="sin_l")
            cos_t = tabp.tile([128, NT, T_TILE], F32, name="cos_t")
            sin_t = tabp.tile([128, NT, T_TILE], F32, name="sin_t")

            def rope_chunk(pos_f_ap, cos_dst, sin_dst, rp, pp, tag):
                """cos/sin [128,512] from f32r positions [1,512] (PE bcast)."""
                pb = pp.tile([128, T_TILE], F32, tag="posbc", bufs=2)
                nc.tensor.matmul(pb[:], onesr_t[:], pos_f_ap,
                                 start=True, stop=True)
                freqs = rp.tile([128, T_TILE], F32, tag=f"fq_{tag}")
                nc.vector.tensor_scalar_mul(freqs[:], pb[:], ivf_t[:])
                kr = rp.tile([128, T_TILE], F32, tag=f"kr_{tag}")
                nc.vector.tensor_scalar(kr[:], freqs[:], 1.0 / TWO_PI,
                                        float(MAGIC), mybir.AluOpType.mult,
                                        mybir.AluOpType.add)
                nc.vector.tensor_scalar_sub(kr[:], kr[:], float(MAGIC))
                red = rp.tile([128, T_TILE], F32, tag=f"rd_{tag}")
                nc.vector.cody_waite_cascade(red[:], freqs[:], kr[:],
                                             CW1, CW2, CW3)
                nc.scalar.activation(sin_dst, red[:],
                                     mybir.ActivationFunctionType.Sin)
                redc = rp.tile([128, T_TILE], F32, tag=f"rc_{tag}")
                nc.vector.add_range_wrap(redc[:], red[:], math.pi / 2.0,
                                         math.pi, TWO_PI)
                nc.scalar.activation(cos_dst, redc[:],
                                     mybir.ActivationFunctionType.Sin)

            # ---------- phase A ----------
            with nc.named_scope("proj_a"), \
                 tc.tile_pool(name="ap_", bufs=1) as ap_, \
                 tc.tile_pool(name="wa", bufs=1) as wap, \
                 tc.tile_pool(name="va", bufs=1) as vap, \
                 tc.tile_pool(name="pa", bufs=3, space="PSUM") as pap, \
                 tc.tile_pool(name="ssp", bufs=1, space="PSUM") as ssp:
                xa = ap_.tile([128, NHI, T_TILE], BF16)
                for hi in range(4):
                    nc.sync.dma_start(xa[:, hi, :], xTl_r[:, hi, :])
                # preload ALL phase-A weights before any collective runs:
                # DMA throughput collapses ~10x while a mesh transfer flies
                wts_kv = []
                for fo in range(5):
                    m = 128 if fo < 4 else 64
                    wt = wap.tile([128, NHI, 128], BF16, name=f"wkv{fo}")
                    wts_kv.append(wt)
                    if fo == 0:
                        for hc in range(4):
                            nc.sync.dma_start(
                                wt[:, hc * 4:(hc + 1) * 4, :m],
                                w_kvaT_r[:, hc * 4:(hc + 1) * 4, :m])
                        for hi in range(4, NHI):
                            nc.sync.dma_start(xa[:, hi, :], xTl_r[:, hi, :])
                    else:
                        nc.sync.dma_start(
                            wt[:, :, :m],
                            w_kvaT_r[:, :, fo * 128:fo * 128 + m])
                wts_qa = []
                for fo in range(NFO_QA):
                    wt = wap.tile([128, NHI, 128], BF16, name=f"wqa{fo}")
                    wts_qa.append(wt)
                    nc.sync.dma_start(
                        wt[:], w_qaT_r[:, :, fo * 128:(fo + 1) * 128])

                # local rope tables (PE bcast + vector + sin)
                posl_f = ap_.tile([1, T_TILE], F32R, name="posl_f")
                nc.vector.tensor_copy(posl_f[:], pos_l_t[:])
                rope_chunk(posl_f[:], cos_l[:], sin_l[:], vap, pap, "loc")

                val_qa = ap_.tile([128, NFO_QA, T_TILE], BF16)
                val_kv = ap_.tile([128, 5, T_TILE], BF16)
                ss_qa = ssp.tile([128, T_TILE], F32, name="ss_qa")
                ss_kv = ssp.tile([128, T_TILE], F32, name="ss_kv")

                def a_proj_fo(proj, fo, wt):
                    m = 128 if not (proj == 0 and fo == 4) else 64
                    bias_t = bkva_t if proj == 0 else bqa_t
                    vdst = val_kv if proj == 0 else val_qa
                    sst = ss_kv if proj == 0 else ss_qa
                    nsq = 4 if proj == 0 else NFO_QA
                    ps = pap.tile([m, T_TILE], F32, tag="acc")
                    for hi in range(NHI):
                        nc.tensor.matmul(
                            ps[:], wt[:, hi, :m], xa[:, hi, :],
                            start=(hi == 0), stop=(hi == NHI - 1))
                    nc.vector.tensor_scalar_add(
                        vdst[:m, fo, :], ps[:], bias_t[:m, fo:fo + 1])
                    if fo < nsq:
                        sq = vap.tile([128, T_TILE], F32R, tag="sq")
                        nc.vector.tensor_tensor(
                            sq[:], vdst[:, fo, :], vdst[:, fo, :],
                            mybir.AluOpType.mult)
                        nc.tensor.matmul(
                            sst[:], ones_t[:], sq[:],
                            start=(fo == 0), stop=(fo == nsq - 1))

                def a_rstd(proj, dst_row):
                    d = KVLR if proj == 0 else QLR
                    sst = ss_kv if proj == 0 else ss_qa
                    ms = vap.tile([128, T_TILE], F32, tag="ms")
                    nc.vector.tensor_scalar(
                        ms[:], sst[:], 1.0 / d, EPS,
                        mybir.AluOpType.mult, mybir.AluOpType.add)
                    rrow = vap.tile([1, T_TILE], BF16, tag="rrow")
                    nc.scalar.activation(
                        rrow[:], ms[0:1, :],
                        mybir.ActivationFunctionType.Abs_reciprocal_sqrt)
                    nc.sync.dma_start(dst_row, rrow[:])

                # ---- kv path first: unblocks the ckv AllGather early
                for fo in range(5):
                    a_proj_fo(0, fo, wts_kv[fo])
                    if fo < 4:
                        nc.sync.dma_start(
                            ckv_sh[fo * 128:(fo + 1) * 128, :],
                            val_kv[:, fo, :])
                a_rstd(0, ckv_sh[NKV:NKV + 1, :])
                # k_pe rope (local tokens, raw) -> ckv_sh rows 512..576
                kpe = val_kv[0:64, 4, :]
                rps = pap.tile([64, T_TILE], F32, tag="rotk", bufs=1)
                nc.tensor.matmul(rps[:], p128_t[0:64, 0:64], kpe,
                                 start=True, stop=True)
                tmp = vap.tile([64, T_TILE], F32, tag="tmpk")
                nc.vector.tensor_tensor(tmp[:], cos_l[0:64, :], kpe,
                                        mybir.AluOpType.mult)
                rot = vap.tile([64, T_TILE], F32, tag="rotk2")
                nc.vector.tensor_tensor(rot[:], sin_l[0:64, :], rps[:],
                                        mybir.AluOpType.mult)
                kro = vap.tile([64, T_TILE], BF16, tag="kro")
                nc.vector.tensor_tensor(kro[:], tmp[:], rot[:],
                                        mybir.AluOpType.add)
                nc.sync.dma_start(ckv_sh[KVLR:KVLR + DR, :], kro[:])
                nc.gpsimd.collective_compute(
                    "AllGather", mybir.AluOpType.bypass,
                    replica_groups=REPLICA_GROUPS,
                    ins=[ckv_sh[:]], outs=[ckv_all[:]])

                # full rope tables (needed by proj_b q-rope)
                posf_f = ap_.tile([1, S], F32R, name="posf_f")
                nc.vector.tensor_copy(posf_f[:], pos_t[:])
                for t in range(NT):
                    tsl = slice(t * T_TILE, (t + 1) * T_TILE)
                    rope_chunk(posf_f[:, tsl], cos_t[:, t, :],
                               sin_t[:, t, :], vap, pap, f"f{t}")

                # ---- q path, in two ring-halves
                for half in range(2):
                    for fo in range(half * QH, (half + 1) * QH):
                        a_proj_fo(1, fo, wts_qa[fo])
                        shr = qa_sh1_r if half == 0 else qa_sh2_r
                        nc.sync.dma_start(shr[:, fo - half * QH, :],
                                          val_qa[:, fo, :])
                    if half == 0:
                        nc.gpsimd.collective_compute(
                            "AllGather", mybir.AluOpType.bypass,
                            replica_groups=REPLICA_GROUPS,
                            ins=[qa_sh1[:]], outs=[qa_all1[:]])
                    else:
                        a_rstd(1, qa_sh2[QH * 128:QH * 128 + 1, :])
                        nc.gpsimd.collective_compute(
                            "AllGather", mybir.AluOpType.bypass,
                            replica_groups=REPLICA_GROUPS,
                            ins=[qa_sh2[:]], outs=[qa_all2[:]])

            # B->C resident tiles (bf16), direct-written by phase B
            kv_res = tc.alloc_tile_pool(name="kv_res", bufs=1)
            kn_sb = kv_res.tile([128, HPC, S], BF16, name="kn_sb")
            vh_sb = kv_res.tile([128, HPC, KB, DV], BF16, name="vh_sb")
            krot_sb = kv_res.tile([64, S], BF16, name="krot_sb")
            qn_sb = kv_res.tile([128, HPC, S], BF16, name="qn_sb")
            qr_sb = kv_res.tile([64, HPC, S], BF16, name="qr_sb")
            rkv_sb = kv_res.tile([128, NT, T_TILE], BF16, name="rkv_sb")
            rqa_sb = kv_res.tile([128, NT, T_TILE], BF16, name="rqa_sb")
            rkvc_raw = kv_res.tile([128, NT, 4], BF16, name="rkvc_raw")
            rkvc_sb = kv_res.tile([128, NT, 4], F32, name="rkvc_sb")

            w_qbT_r = w_qbT.rearrange("(fi p) f -> p fi f", p=128)
            w_kvb_nT_r = w_kvb_nT.rearrange("(fi p) f -> p fi f", p=128)
            w_kvb_vT_r = w_kvb_vT.rearrange("(fi p) f -> p fi f", p=128)
            w_oT_r = w_oT.rearrange("(fs p) hid -> p fs hid", p=128)

            cwp = tc.alloc_tile_pool(name="cw", bufs=1)
            wo_sb = cwp.tile([128, HPC, HID], BF16, name="wo_sb")
            masks_t = cwp.tile([128, 4, T_TILE], BF16, name="masks_t")

            # ---------- phase B ----------
            with nc.named_scope("proj_b"), \
                 tc.tile_pool(name="wb", bufs=1) as wbp, \
                 tc.tile_pool(name="rhb", bufs=2) as rhbp, \
                 tc.tile_pool(name="evb", bufs=2) as evbp, \
                 tc.tile_pool(name="pb", bufs=2, space="PSUM") as pbp, \
                 tc.tile_pool(name="bc", bufs=2, space="PSUM") as bcp:
                wv_t = wbp.tile([128, NFO_KV, HPC * DV], BF16, name="wv_t")
                nc.sync.dma_start(wv_t[:], w_kvb_vT_r[:])
                wkn_t = wbp.tile([128, NFO_KV, HPC * DN], BF16, name="wkn_t")
                nc.sync.dma_start(wkn_t[:], w_kvb_nT_r[:])
                wqb_t = wbp.tile([128, NFO_QA, HPC * QD], BF16, name="wqb_t")
                nc.sync.dma_start(wqb_t[:], w_qbT_r[:])
                nc.sync.dma_start(masks_t[:],
                                  masks.rearrange("j p t -> p j t"))
                nc.sync.dma_start(wo_sb[:], w_oT_r[:])

                # ---- kv_b nope + v for all tiles (dep: ckv AllGather)
                for t in range(NT):
                    tsl = slice(t * T_TILE, (t + 1) * T_TILE)
                    ckv_rhs = rhbp.tile([128, NFO_KV, T_TILE], BF16,
                                        tag="ckvrhs")
                    for f in range(NFO_KV):
                        nc.gpsimd.dma_start(
                            ckv_rhs[:, f, :],
                            ckv_all[t, f * 128:(f + 1) * 128, :])
                    nc.gpsimd.dma_start(
                        krot_sb[:, tsl], ckv_all[t, KVLR:KVLR + DR, :])
                    # rstd_kv row -> bcast [128,T] and column [128,4] forms
                    rrow = rhbp.tile([1, T_TILE], BF16, tag="rrow")
                    nc.gpsimd.dma_start(rrow[:], ckv_all[t, NKV:NKV + 1, :])
                    nc.gpsimd.dma_start(
                        rkvc_raw[:, t, :],
                        ckv_all[t, NKV, :].rearrange("(ts p) -> p ts", p=128))
                    nc.vector.tensor_copy(rkvc_sb[:, t, :], rkvc_raw[:, t, :])
                    rbc = bcp.tile([128, T_TILE], F32, tag="rbc")
                    nc.tensor.matmul(rbc[:], onesrb_t[:], rrow[:],
                                     start=True, stop=True)
                    nc.vector.tensor_copy(rkv_sb[:, t, :], rbc[:])
                    for fo in range(HPC):
                        ps = pbp.tile([128, T_TILE], F32, tag="pb")
                        for fi in range(NFO_KV):
                            nc.tensor.matmul(
                                ps[:], wkn_t[:, fi, fo * 128:(fo + 1) * 128],
                                ckv_rhs[:, fi, :],
                                start=(fi == 0), stop=(fi == NFO_KV - 1))
                        nc.vector.tensor_tensor(
                            kn_sb[:, fo, tsl], ps[:], rkv_sb[:, t, :],
                            mybir.AluOpType.mult)
                    for ts in range(T_TILE // 128):
                        kb = t * 4 + ts
                        ps = pbp.tile([128, HPC * DV], F32, tag="pb")
                        for fi in range(NFO_KV):
                            nc.tensor.matmul(
                                ps[:],
                                ckv_rhs[:, fi, ts * 128:(ts + 1) * 128],
                                wv_t[:, fi, :],
                                start=(fi == 0), stop=(fi == NFO_KV - 1))
                        nc.vector.tensor_scalar_mul(
                            vh_sb[:, :, kb, :],
                            ps[:].rearrange("p (h d) -> p h d", h=HPC),
                            rkvc_sb[:, t, ts:ts + 1])

                # ---- q_b + q rope for all tiles (dep: qa AllGathers)
                for t in range(NT):
                    tsl = slice(t * T_TILE, (t + 1) * T_TILE)
                    qa_rhs = rhbp.tile([128, NFO_QA, T_TILE], BF16,
                                       tag="qarhs")
                    for f in range(NFO_QA):
                        src = qa_all1 if f < QH else qa_all2
                        nc.gpsimd.dma_start(
                            qa_rhs[:, f, :],
                            src[t, (f % QH) * 128:(f % QH + 1) * 128, :])
                    rrow = rhbp.tile([1, T_TILE], BF16, tag="rrow")
                    nc.gpsimd.dma_start(
                        rrow[:], qa_all2[t, QH * 128:QH * 128 + 1, :])
                    rbc = bcp.tile([128, T_TILE], F32, tag="rbc")
                    nc.tensor.matmul(rbc[:], onesrb_t[:], rrow[:],
                                     start=True, stop=True)
                    nc.vector.tensor_copy(rqa_sb[:, t, :], rbc[:])
                    for fo in range(NQB):
                        ps = pbp.tile([128, T_TILE], F32, tag="pb")
                        for fi in range(NFO_QA):
                            nc.tensor.matmul(
                                ps[:], wqb_t[:, fi, fo * 128:(fo + 1) * 128],
                                qa_rhs[:, fi, :],
                                start=(fi == 0), stop=(fi == NFO_QA - 1))
                        if fo < HPC:  # nope
                            nc.vector.tensor_tensor(
                                qn_sb[:, fo, tsl], ps[:], rqa_sb[:, t, :],
                                mybir.AluOpType.mult)
                        else:  # rope pair: rows = heads (2j, 2j+1)
                            j = fo - HPC
                            qpe = evbp.tile([128, T_TILE], BF16, tag="qpe")
                            nc.vector.tensor_tensor(
                                qpe[:], ps[:], rqa_sb[:, t, :],
                                mybir.AluOpType.mult)
                            rps = pbp.tile([128, T_TILE], F32, tag="rot",
                                           bufs=1)
                            nc.tensor.matmul(rps[:], p128_t[:], qpe[:],
                                             start=True, stop=True)
                            tmp = evbp.tile([128, T_TILE], F32, tag="tmp")
                            nc.vector.tensor_tensor(
                                tmp[:], cos_t[:, t, :], qpe[:],
                                mybir.AluOpType.mult)
                            rot = evbp.tile([128, T_TILE], F32, tag="rot2")
                            nc.vector.tensor_tensor(
                                rot[:], sin_t[:, t, :], rps[:],
                                mybir.AluOpType.mult)
                            nc.vector.tensor_tensor(
                                qr_sb[:, 2 * j, tsl], tmp[0:64, :],
                                rot[0:64, :], mybir.AluOpType.add)
                            nc.vector.tensor_tensor(
                                qr_sb[:, 2 * j + 1, tsl], tmp[64:128, :],
                                rot[64:128, :], mybir.AluOpType.add)

            if debug:
                nc.sync.dma_start(dbg_kn[:], kn_sb[:])
                nc.sync.dma_start(dbg_v.rearrange("kb p h d -> p h kb d"),
                                  vh_sb[:])
                nc.sync.dma_start(dbg_krot[:], krot_sb[:])
                nc.sync.dma_start(dbg_qn[:], qn_sb[:])
                nc.sync.dma_start(dbg_qr[:], qr_sb[:])

            # ---------- phase C: attention + fused o_proj ----------
            with nc.named_scope("attn"), \
                 tc.tile_pool(name="pt", bufs=4) as ptp, \
                 tc.tile_pool(name="acc", bufs=2) as accp, \
                 tc.tile_pool(name="ao", bufs=2) as aop, \
                 tc.tile_pool(name="oe", bufs=3) as oep, \
                 tc.tile_pool(name="sps", bufs=2, space="PSUM") as spsp, \
                 tc.tile_pool(name="avs", bufs=2, space="PSUM") as avsp, \
                 tc.tile_pool(name="lps", bufs=1, space="PSUM") as lpsp, \
                 tc.tile_pool(name="pos_", bufs=2, space="PSUM") as posp:
                for qt in range(NT):
                    qsl = slice(qt * T_TILE, (qt + 1) * T_TILE)
                    at_full = aop.tile([128, HPC, T_TILE], BF16, tag="atf")
                    nkb = 4 * qt + 4
                    for h in range(HPC):
                        av_ps = avsp.tile([128, T_TILE], F32, tag="av")
                        acc = accp.tile([128, T_TILE], F32R, tag="acc")
                        for kb in range(nkb):
                            sps = spsp.tile([128, T_TILE], F32, tag="s")
                            nc.tensor.matmul(
                                sps[:],
                                kn_sb[:, h, kb * 128:(kb + 1) * 128],
                                qn_sb[:, h, qsl], start=True, stop=False)
                            nc.tensor.matmul(
                                sps[:], krot_sb[:, kb * 128:(kb + 1) * 128],
                                qr_sb[:, h, qsl], start=False, stop=True)
                            pt = ptp.tile([128, T_TILE], BF16, tag="p")
                            nc.scalar.activation(
                                pt[:], sps[:],
                                mybir.ActivationFunctionType.Exp, scale=SCALE)
                            j = kb - 4 * qt
                            if j >= 0:
                                nc.vector.tensor_tensor(
                                    pt[:], pt[:], masks_t[:, j, :],
                                    mybir.AluOpType.mult)
                            if kb == 0:
                                nc.vector.tensor_copy(acc[:], pt[:])
                            else:
                                nc.vector.tensor_tensor(
                                    acc[:], acc[:], pt[:],
                                    mybir.AluOpType.add)
                            nc.tensor.matmul(
                                av_ps[:], vh_sb[:, h, kb, :], pt[:],
                                start=(kb == 0), stop=(kb == nkb - 1))
                        l_bc = lpsp.tile([128, T_TILE], F32, tag="l")
                        nc.tensor.matmul(l_bc[:], ones_t[:], acc[:],
                                         start=True, stop=True)
                        rsq = ptp.tile([128, T_TILE], F32, tag="rsq")
                        nc.scalar.activation(
                            rsq[:], l_bc[:],
                            mybir.ActivationFunctionType.Abs_reciprocal_sqrt)
                        rec = ptp.tile([128, T_TILE], F32, tag="rec")
                        nc.vector.tensor_tensor(rec[:], rsq[:], rsq[:],
                                                mybir.AluOpType.mult)
                        nc.vector.tensor_tensor(
                            at_full[:, h, :], av_ps[:], rec[:],
                            mybir.AluOpType.mult)
                        if debug:
                            nc.sync.dma_start(dbg_attn[:, h, qsl],
                                              at_full[:, h, :])
                    # fused o_proj for this q-tile (evict+out on gpsimd)
                    for ts in range(T_TILE // 128):
                        tok0 = qt * T_TILE + ts * 128
                        for ho in range(HID // T_TILE):
                            ps = posp.tile([128, T_TILE], F32, tag="po")
                            for fs in range(HPC):
                                nc.tensor.matmul(
                                    ps[:],
                                    at_full[:, fs, ts * 128:(ts + 1) * 128],
                                    wo_sb[:, fs,
                                          ho * T_TILE:(ho + 1) * T_TILE],
                                    start=(fs == 0), stop=(fs == HPC - 1))
                            oe = oep.tile([128, T_TILE], BF16, tag="oe")
                            nc.scalar.activation(
                                oe[:], ps[:],
                                mybir.ActivationFunctionType.Copy)
                            nc.gpsimd.dma_start(
                                out[tok0:tok0 + 128,
                                    ho * T_TILE:(ho + 1) * T_TILE],
                                oe[:])
            cwp.release()
            kv_res.release()
            tabp.release()

    nc.compile()
    _BUILD_CACHE[key] = nc
    return nc


def _host_consts():
    ivf = (1.0 / (ROPE_BASE ** (np.arange(0, DR, 2, dtype=np.float64) / DR)))
    ivf = ivf.astype(np.float32)                       # [32]
    inv_freq128 = np.tile(ivf, 4).reshape(128, 1)

    rot = np.zeros((DR, DR), np.float32)               # rot(x) = P @ x
    for d in range(32):
        rot[d, d + 32] = -1.0
        rot[d + 32, d] = 1.0
    rotT = rot.T
    p128 = np.zeros((128, 128), np.float32)
    p128[:64, :64] = rotT
    p128[64:, 64:] = rotT

    kk = np.arange(128)[None, :, None]                 # [1,128,1]
    jj = np.arange(4)[:, None, None]                   # [4,1,1]
    qq = np.arange(T_TILE)[None, None, :]              # [1,1,512]
    masks = ((jj * 128 + kk) <= qq).astype(np.float32)  # [4,128,512]

    return inv_freq128, p128, masks


LAST_RES = None


def kernel(_debug=False, **inputs):
    hidden_states = np.asarray(inputs["hidden_states"], np.float32)
    position_ids = np.asarray(inputs["position_ids"])
    W_qa = np.asarray(inputs["W_qa"], np.float32)
    b_qa = np.asarray(inputs["b_qa"], np.float32)
    w_qa_ln = np.asarray(inputs["w_qa_ln"], np.float32)
    W_qb = np.asarray(inputs["W_qb"], np.float32)
    W_kva = np.asarray(inputs["W_kva"], np.float32)
    b_kva = np.asarray(inputs["b_kva"], np.float32)
    w_kva_ln = np.asarray(inputs["w_kva_ln"], np.float32)
    W_kvb = np.asarray(inputs["W_kvb"], np.float32)
    W_o = np.asarray(inputs["W_o"], np.float32)

    nc = build_kernel(debug=_debug)

    inv_freq128, p128, masks = _host_consts()

    w_qaT = np.ascontiguousarray(W_qa.T)
    w_kvaT = np.ascontiguousarray(W_kva.T)
    # fold the rmsnorm scale into the B-projections
    W_qb_f = W_qb * w_qa_ln[None, :]
    W_kvb_f = W_kvb * w_kva_ln[None, :]
    W_qb_h = W_qb_f.reshape(NH, QD, QLR)
    W_kvb_h = W_kvb_f.reshape(NH, DN + DV, KVLR)
    b_qa_t = np.ascontiguousarray(b_qa.reshape(NFO_QA, 128).T)
    b_kva_p = np.zeros(640, np.float32)
    b_kva_p[:NKV] = b_kva
    b_kva_t = np.ascontiguousarray(b_kva_p.reshape(5, 128).T)
    ones128 = np.ones((128, 128), np.float32)
    ones_row = np.ones((1, 128), np.float32)

    in_maps = []
    for c in range(N_CORES):
        b = c // TPG
        g = c % TPG
        hs = list(range(g * HPC, (g + 1) * HPC))
        # q_b columns: nope blocks by head then rope blocks by head
        qb_nope = np.concatenate([W_qb_h[h, :DN, :] for h in hs], 0)
        qb_rope = np.concatenate([W_qb_h[h, DN:, :] for h in hs], 0)
        w_qbT = np.ascontiguousarray(np.concatenate([qb_nope, qb_rope], 0).T)
        w_kvb_nT = np.ascontiguousarray(
            np.concatenate([W_kvb_h[h, :DN, :] for h in hs], 0).T)
        w_kvb_vT = np.ascontiguousarray(
            np.concatenate([W_kvb_h[h, DN:, :] for h in hs], 0).T)
        w_oT = np.ascontiguousarray(
            W_o[:, g * HPC * DV:(g + 1) * HPC * DV].T)
        pos_b = position_ids[b].astype(np.int32)
        in_maps.append({
            "xTl": np.ascontiguousarray(
                hidden_states[b].T[:, g * T_TILE:(g + 1) * T_TILE]
                ).astype(BF16NP),
            "w_qaT": w_qaT.astype(BF16NP),
            "w_kvaT": w_kvaT.astype(BF16NP),
            "w_qbT": w_qbT.astype(BF16NP),
            "w_kvb_nT": w_kvb_nT.astype(BF16NP),
            "w_kvb_vT": w_kvb_vT.astype(BF16NP),
            "w_oT": w_oT.astype(BF16NP),
            "b_qa": b_qa_t, "b_kva": b_kva_t,
            "pos": np.ascontiguousarray(pos_b.reshape(1, S)),
            "pos_l": np.ascontiguousarray(
                pos_b[g * T_TILE:(g + 1) * T_TILE].reshape(1, T_TILE)),
            "inv_freq": inv_freq128,
            "p128": p128.astype(BF16NP),
            "ones128": ones128, "ones_row": ones_row,
            "masks": masks.astype(BF16NP),
        })

    res = run_bass_kernel_spmd(nc, in_maps, list(range(N_CORES)))
    global LAST_RES
    LAST_RES = res

    out = np.zeros((B, S, HID), np.float32)
    for c in range(N_CORES):
        out[c // TPG] += res.results[c]["out"].astype(np.float32)
    return out


if __name__ == "__main__":
    import time
    t0 = time.time()
    build_kernel()
    print(f"build+compile: {time.time()-t0:.1f}s")
